# revision 21
# baseline (speedup 1.0000x reference)
"""Multi-head attention (B=8, T=2048, D=512, H=8) on 8 TRN2 NeuronCores.

Sharding: data-parallel over batch — one batch element per core, no
collectives. Host-side prep (part of shard/unshard): transpose x inputs to
[D, T], cast matmul operands to bf16, pass (1 - mask)^T chunk-major, and
transpose the per-core output y^T back to [T, D].

Per-core algorithm ("transposed flash", everything in one PE tiling mode):
  P1: Q^T = Wq x^T, K^T = Wk x^T (padded per-head into zero-padded 128-row
      tiles), V = x Wv^T (augmented with a ones column per head for the
      softmax denominator).
  P2: per (q-block, head, t2-chunk):
        S^T[t2,q] = Kpad_h^T.T @ Q^T          (PSUM, scale deferred)
        P_raw     = exp(S^T / 8)              (ScalarE, PSUM -> SBUF bf16)
        P         = P_raw * (1-mask)^T        (VectorE; equals reference's
                                               where(mask,-inf) + where(mask,0)
                                               since exp(-1e4) == 0 in f32)
        O_aug^T  += Vaug_h.T @ P              (PSUM accum; row 64 = denom)
      epilogue: recip(denom) -> broadcast -> O^T = O_aug^T[0:64] * recip.
  P3: y^T = Wo^T.T @ O^T (+bo), DMA out.

Biases bq, bk, bo are applied (per-partition fused adds); bv via a
broadcast add on V eviction. No max-subtraction in softmax: scores are
O(6) so exp is safe in f32, matching the reference to ~bf16 accuracy.
"""

import numpy as np
import ml_dtypes

B, T, FDIM, H = 8, 2048, 512, 8
DK = FDIM // H          # 64
NFT = FDIM // 128       # 4 fo-tiles
NCH = T // 128          # 16 t2-chunks
QB = 2                  # q blocks
QBS = T // QB           # 1024
N_CORES = 8

BF16 = ml_dtypes.bfloat16

_cache = {}


def _build_nc():
    import concourse.bass as bass
    import concourse.mybir as mybir
    from concourse import bacc, tile

    f32 = mybir.dt.float32
    bf16 = mybir.dt.bfloat16
    Exp = mybir.ActivationFunctionType.Exp
    Alu = mybir.AluOpType

    nc = bacc.Bacc("TRN2", target_bir_lowering=False, debug=False,
                   num_devices=N_CORES)

    # DRAM I/O (per-core shard shapes)
    xqT = nc.dram_tensor("xqT", [FDIM, T], bf16, kind="ExternalInput")
    xkT = nc.dram_tensor("xkT", [FDIM, T], bf16, kind="ExternalInput")
    xvT = nc.dram_tensor("xvT", [FDIM, T], bf16, kind="ExternalInput")
    wqT = nc.dram_tensor("wqT", [FDIM, FDIM], bf16, kind="ExternalInput")
    wkT = nc.dram_tensor("wkT", [FDIM, FDIM], bf16, kind="ExternalInput")
    wvT = nc.dram_tensor("wvT", [FDIM, FDIM], bf16, kind="ExternalInput")
    woT = nc.dram_tensor("woT", [FDIM, FDIM], bf16, kind="ExternalInput")
    bq = nc.dram_tensor("bq", [FDIM], f32, kind="ExternalInput")
    bk = nc.dram_tensor("bk", [FDIM], f32, kind="ExternalInput")
    bv = nc.dram_tensor("bv", [FDIM], f32, kind="ExternalInput")
    bo = nc.dram_tensor("bo", [FDIM], f32, kind="ExternalInput")
    mbar = nc.dram_tensor("mbar", [NCH, 128, T], bf16, kind="ExternalInput")
    yT = nc.dram_tensor("yT", [FDIM, T], f32, kind="ExternalOutput")
    # DRAM bounce rows for partition-broadcasting softmax reciprocals
    rscratch = nc.dram_tensor("rscratch", [QB * H, QBS], f32)

    import os
    dbg = os.environ.get("KERNEL_DEBUG_DUMPS") == "1"
    if dbg:
        dbg_qt = nc.dram_tensor("dbg_qt", [128, T], bf16, kind="ExternalOutput")
        dbg_kp = nc.dram_tensor("dbg_kp", [2, 128, T], bf16, kind="ExternalOutput")
        dbg_va = nc.dram_tensor("dbg_va", [128, H * (DK + 1)], bf16, kind="ExternalOutput")
        dbg_pm = nc.dram_tensor("dbg_pm", [128, QBS], bf16, kind="ExternalOutput")
        dbg_osb = nc.dram_tensor("dbg_osb", [64, QBS], bf16, kind="ExternalOutput")
        dbg_rb = nc.dram_tensor("dbg_rb", [2, QBS], f32, kind="ExternalOutput")

    with tile.TileContext(nc) as tc:
        with (
            tc.tile_pool(name="consts", bufs=1) as consts,
            tc.tile_pool(name="qt", bufs=1) as qt_pool,
            tc.tile_pool(name="kpad", bufs=1) as kpad_pool,
            tc.tile_pool(name="vaug", bufs=1) as vaug_pool,
            tc.tile_pool(name="osb", bufs=1) as osb_pool,
            tc.tile_pool(name="ysb", bufs=2) as ysb_pool,
        ):
            # ---- consts: weights + biases ----
            wq_sb = [consts.tile([128, FDIM], bf16, tag=f"wq{fc}", name=f"wq{fc}") for fc in range(4)]
            wk_sb = [consts.tile([128, FDIM], bf16, tag=f"wk{fc}", name=f"wk{fc}") for fc in range(4)]
            wv_sb = [consts.tile([128, FDIM], bf16, tag=f"wv{fc}", name=f"wv{fc}") for fc in range(4)]
            wo_sb = [consts.tile([64, FDIM], bf16, tag=f"wo{h}", name=f"wo{h}") for h in range(H)]
            for fc in range(4):
                nc.sync.dma_start(out=wq_sb[fc][:], in_=wqT[fc * 128:(fc + 1) * 128, :])
                nc.sync.dma_start(out=wk_sb[fc][:], in_=wkT[fc * 128:(fc + 1) * 128, :])
                nc.sync.dma_start(out=wv_sb[fc][:], in_=wvT[fc * 128:(fc + 1) * 128, :])
            for h in range(H):
                nc.sync.dma_start(out=wo_sb[h][:], in_=woT[h * 64:(h + 1) * 64, :])

            bq_sb = consts.tile([128, NFT], f32, tag="bq", name="bq")
            bk_sb = consts.tile([128, NFT], f32, tag="bk", name="bk")
            bo_sb = consts.tile([128, NFT], f32, tag="bo", name="bo")
            for b_dram, b_t in ((bq, bq_sb), (bk, bk_sb), (bo, bo_sb)):
                nc.sync.dma_start(out=b_t[:], in_=b_dram.ap().rearrange("(j p) -> p j", p=128))
            bv_bcast = consts.tile([128, FDIM], f32, tag="bv_bcast", name="bv_bcast")
            nc.sync.dma_start(
                out=bv_bcast[:],
                in_=bv.ap().rearrange("(a f) -> a f", a=1).to_broadcast([128, FDIM]))

            # ---- persistent activation tiles ----
            qT_sb = [qt_pool.tile([128, T], bf16, tag=f"qT{j}", name=f"qT{j}") for j in range(NFT)]
            kT_sb = [kpad_pool.tile([128, T], bf16, tag=f"kT{j}", name=f"kT{j}") for j in range(NFT)]
            vaug = [vaug_pool.tile([128, H * (DK + 1)], bf16, tag=f"va{tt}", name=f"va{tt}")
                    for tt in range(NCH)]
            # ones column per head in V_aug
            for tt in range(NCH):
                va = vaug[tt][:].rearrange("p (h d) -> p h d", d=DK + 1)
                nc.vector.memset(va[:, :, DK:DK + 1], 1.0)

            o_sb = {}
            for qb in range(QB):
                for h in range(H):
                    o_sb[(qb, h)] = osb_pool.tile([64, QBS], bf16, tag=f"o{qb}_{h}", name=f"o{qb}_{h}")

            # ================= P1: projections =================
            with (
                tc.tile_pool(name="xt", bufs=5) as xt_pool,
                tc.tile_pool(name="pp", bufs=2, space="PSUM") as pp_pool,
            ):
                def load_xT(xT_dram):
                    tiles = []
                    for fc in range(4):
                        xt = xt_pool.tile([128, T], bf16, tag="xt", name="xt")
                        nc.sync.dma_start(out=xt[:], in_=xT_dram[fc * 128:(fc + 1) * 128, :])
                        tiles.append(xt)
                    return tiles

                # Q^T[fo, t] and K^T (into padded per-head tiles)
                for name, xT_dram, w_sb, b_t in (("q", xqT, wq_sb, bq_sb),
                                                 ("k", xkT, wk_sb, bk_sb)):
                    xts = load_xT(xT_dram)
                    for j in range(NFT):
                        for s in range(4):
                            ps = pp_pool.tile([128, 512], mybir.dt.float32, tag="pp", name="pp")
                            for fc in range(4):
                                nc.tensor.matmul(
                                    ps[:],
                                    wq_sb[fc][:, j * 128:(j + 1) * 128] if name == "q"
                                    else wk_sb[fc][:, j * 128:(j + 1) * 128],
                                    xts[fc][:, s * 512:(s + 1) * 512],
                                    start=(fc == 0), stop=(fc == 3),
                                )
                            sl = slice(s * 512, (s + 1) * 512)
                            dst = qT_sb[j] if name == "q" else kT_sb[j]
                            nc.vector.tensor_scalar_add(
                                dst[:, sl], ps[:], b_t[:, j:j + 1])

                # V[t, fo] into augmented tiles (+bv broadcast add)
                xts = load_xT(xvT)
                for tt in range(NCH):
                    ps = pp_pool.tile([128, 512], mybir.dt.float32, tag="pp", name="pp")
                    for fc in range(4):
                        nc.tensor.matmul(
                            ps[:],
                            xts[fc][:, tt * 128:(tt + 1) * 128],
                            wv_sb[fc][:],
                            start=(fc == 0), stop=(fc == 3),
                        )
                    va = vaug[tt][:].rearrange("p (h d) -> p h d", d=DK + 1)
                    nc.vector.scalar_tensor_tensor(
                        out=va[:, :, 0:DK],
                        in0=ps[:].rearrange("p (h d) -> p h d", d=DK),
                        scalar=1.0,
                        in1=bv_bcast[:].rearrange("p (h d) -> p h d", d=DK),
                        op0=Alu.mult, op1=Alu.add,
                    )

            if dbg:
                nc.sync.dma_start(out=dbg_qt.ap(), in_=qT_sb[0][:])
                nc.sync.dma_start(out=dbg_kp.ap()[0], in_=kT_sb[0][:])
                nc.sync.dma_start(out=dbg_kp.ap()[1], in_=kT_sb[1][:])
                nc.sync.dma_start(out=dbg_va.ap(), in_=vaug[0][:])

            # ================= P2 + P3: attention =================
            with (
                tc.tile_pool(name="mask", bufs=16) as mask_pool,
                tc.tile_pool(name="praw", bufs=3) as praw_pool,
                tc.tile_pool(name="pm", bufs=3) as pm_pool,
                tc.tile_pool(name="rb", bufs=2) as rb_pool,
                tc.tile_pool(name="sps", bufs=2, space="PSUM") as sps_pool,
                tc.tile_pool(name="ops", bufs=2, space="PSUM") as ops_pool,
            ):
                yps_pool = sps_pool  # P3 reuses the score PSUM slots (tag "s")
                for qb in range(QB):
                    qsl = slice(qb * QBS, (qb + 1) * QBS)
                    mask_t = []
                    for c in range(NCH):
                        mt = mask_pool.tile([128, QBS], bf16, tag="mask", name="mask")
                        nc.sync.dma_start(out=mt[:], in_=mbar[c, :, qsl])
                        mask_t.append(mt)

                    # head pairs: even head lives on partitions 0-63, odd on
                    # 64-127 of fo-tile j, so their score matmuls use array
                    # tiles (64,128)@row-0 and @row-64 and run concurrently.
                    for hp in range(H // 2):
                        j = hp
                        heads = (2 * hp, 2 * hp + 1)
                        o_ps = {}
                        for hi, h in enumerate(heads):
                            o_ps[h] = ops_pool.tile([DK + 1, QBS], mybir.dt.float32,
                                                    tag=f"o{hi}", bufs=1, name=f"o{hi}")
                        for c in range(NCH):
                            s_ps = {}
                            for hi, h in enumerate(heads):
                                s_ps[h] = sps_pool.tile([128, QBS], mybir.dt.float32,
                                                        tag="s", name="s")
                            for s in range(2):
                                for hi, h in enumerate(heads):
                                    pr = slice(hi * 64, hi * 64 + 64)
                                    nc.tensor.matmul(
                                        s_ps[h][:, s * 512:(s + 1) * 512],
                                        kT_sb[j][pr, c * 128:(c + 1) * 128],
                                        qT_sb[j][pr, qb * QBS + s * 512: qb * QBS + (s + 1) * 512],
                                        start=True, stop=True,
                                    )
                            for hi, h in enumerate(heads):
                                p_raw = praw_pool.tile([128, QBS], bf16, tag="praw", name="praw")
                                nc.scalar.activation(p_raw[:], s_ps[h][:], Exp,
                                                     bias=0.0, scale=0.125)
                                p_m = pm_pool.tile([128, QBS], bf16, tag="pm", name="pm")
                                # split mask-mult between DVE and GpSimd
                                eng = nc.gpsimd if (c % 3 == 2) else nc.vector
                                eng.tensor_mul(p_m[:], p_raw[:], mask_t[c][:])
                                if dbg and qb == 0 and h == 0 and c == 0:
                                    nc.sync.dma_start(out=dbg_pm.ap(), in_=p_m[:])
                                for s in range(2):
                                    nc.tensor.matmul(
                                        o_ps[h][:, s * 512:(s + 1) * 512],
                                        vaug[c][:, h * (DK + 1):(h + 1) * (DK + 1)],
                                        p_m[:, s * 512:(s + 1) * 512],
                                        start=(c == 0), stop=(c == NCH - 1),
                                    )
                        # epilogue: divide by denominator (row DK of o_ps).
                        # reciprocal is ~8 cyc/elem/lane, so split the [1,1024]
                        # row across 8 partitions via SBUF->SBUF DMA first; a
                        # DRAM bounce row broadcasts it across partitions 0-63.
                        for h in heads:
                            rb = rb_pool.tile([128, QBS], mybir.dt.float32, tag="rb", name="rb")
                            rbd = rb_pool.tile([128, QBS], mybir.dt.float32, tag="rbd", name="rbd")
                            rbs = rb_pool.tile([8, QBS // 8], mybir.dt.float32, tag="rbs", name="rbs")
                            rbr = rb_pool.tile([8, QBS // 8], mybir.dt.float32, tag="rbr", name="rbr")
                            nc.vector.tensor_copy(rbd[64:65, :], o_ps[h][DK:DK + 1, :])
                            nc.sync.dma_start(out=rbs[:], in_=rbd[64:65, :])
                            nc.vector.reciprocal(rbr[:], rbs[:])
                            rrow = rscratch.ap()[qb * H + h: qb * H + h + 1, :]
                            nc.sync.dma_start(out=rrow, in_=rbr[:])
                            nc.sync.dma_start(out=rb[0:64, :],
                                              in_=rrow.to_broadcast([64, QBS]))
                            nc.vector.tensor_mul(o_sb[(qb, h)][:], o_ps[h][0:DK, :],
                                                 rb[0:64, :])
                            if dbg and qb == 0 and h == 0:
                                nc.sync.dma_start(out=dbg_rb.ap()[0:1, :], in_=rb[64:65, :])
                                nc.sync.dma_start(out=dbg_rb.ap()[1:2, :], in_=rbd[64:65, :])
                                nc.sync.dma_start(out=dbg_osb.ap(), in_=o_sb[(qb, h)][:])

                    # P3: output projection for this q block
                    for i in range(NFT):
                        y_ps = yps_pool.tile([128, QBS], mybir.dt.float32, tag="s", name="y")
                        for s in range(2):
                            for h in range(H):
                                nc.tensor.matmul(
                                    y_ps[:, s * 512:(s + 1) * 512],
                                    wo_sb[h][:, i * 128:(i + 1) * 128],
                                    o_sb[(qb, h)][:, s * 512:(s + 1) * 512],
                                    start=(h == 0), stop=(h == H - 1),
                                )
                        y_sb = ysb_pool.tile([128, QBS], mybir.dt.float32, tag="ysb", name="ysb")
                        nc.vector.tensor_scalar_add(y_sb[:], y_ps[:], bo_sb[:, i:i + 1])
                        nc.sync.dma_start(out=yT[i * 128:(i + 1) * 128, qsl], in_=y_sb[:])

    nc.compile()
    return nc


def _get_nc():
    if "nc" not in _cache:
        _cache["nc"] = _build_nc()
    return _cache["nc"]


def _make_in_maps(inputs):
    query = np.asarray(inputs["query"], np.float32)
    key = np.asarray(inputs["key"], np.float32)
    value = np.asarray(inputs["value"], np.float32)
    mask = np.asarray(inputs["mask"], bool)
    shared = {
        "wqT": np.ascontiguousarray(np.asarray(inputs["Wq"], np.float32).T).astype(BF16),
        "wkT": np.ascontiguousarray(np.asarray(inputs["Wk"], np.float32).T).astype(BF16),
        "wvT": np.ascontiguousarray(np.asarray(inputs["Wv"], np.float32).T).astype(BF16),
        "woT": np.ascontiguousarray(np.asarray(inputs["Wo"], np.float32).T).astype(BF16),
        "bq": np.asarray(inputs["bq"], np.float32),
        "bk": np.asarray(inputs["bk"], np.float32),
        "bv": np.asarray(inputs["bv"], np.float32),
        "bo": np.asarray(inputs["bo"], np.float32),
    }
    in_maps = []
    for b in range(N_CORES):
        m = dict(shared)
        m["xqT"] = np.ascontiguousarray(query[b].T).astype(BF16)
        m["xkT"] = np.ascontiguousarray(key[b].T).astype(BF16)
        m["xvT"] = np.ascontiguousarray(value[b].T).astype(BF16)
        mb = (~mask[b]).T.astype(BF16)          # (1 - mask)^T, [t2, q]
        m["mbar"] = np.ascontiguousarray(mb.reshape(NCH, 128, T))
        in_maps.append(m)
    return in_maps


def run(inputs, trace=False, **kwargs):
    from concourse.bass_utils import run_bass_kernel_spmd
    nc = _get_nc()
    res = run_bass_kernel_spmd(nc, _make_in_maps(inputs),
                               core_ids=list(range(N_CORES)),
                               trace=trace, **kwargs)
    y = np.stack([np.asarray(res.results[b]["yT"], np.float32).T
                  for b in range(N_CORES)])
    return y, res


def kernel(**inputs) -> np.ndarray:
    y, _ = run(inputs, trace=False)
    return y


# revision 23
# speedup vs baseline: 1.4037x; 1.4037x over previous
"""Multi-head attention (B=8, T=2048, D=512, H=8) on 8 TRN2 NeuronCores.

Sharding: data-parallel over batch — one batch element per core, no
collectives. Host-side prep (part of shard/unshard): transpose x inputs to
[D, T], cast matmul operands to bf16, pass (1 - mask)^T chunk-major, and
transpose the per-core output y^T back to [T, D].

Per-core algorithm ("transposed flash", everything in one PE tiling mode):
  P1: Q^T = Wq x^T, K^T = Wk x^T (padded per-head into zero-padded 128-row
      tiles), V = x Wv^T (augmented with a ones column per head for the
      softmax denominator).
  P2: per (q-block, head, t2-chunk):
        S^T[t2,q] = Kpad_h^T.T @ Q^T          (PSUM, scale deferred)
        P_raw     = exp(S^T / 8)              (ScalarE, PSUM -> SBUF bf16)
        P         = P_raw * (1-mask)^T        (VectorE; equals reference's
                                               where(mask,-inf) + where(mask,0)
                                               since exp(-1e4) == 0 in f32)
        O_aug^T  += Vaug_h.T @ P              (PSUM accum; row 64 = denom)
      epilogue: recip(denom) -> broadcast -> O^T = O_aug^T[0:64] * recip.
  P3: y^T = Wo^T.T @ O^T (+bo), DMA out.

Biases bq, bk, bo are applied (per-partition fused adds); bv via a
broadcast add on V eviction. No max-subtraction in softmax: scores are
O(6) so exp is safe in f32, matching the reference to ~bf16 accuracy.
"""

import numpy as np
import ml_dtypes

B, T, FDIM, H = 8, 2048, 512, 8
DK = FDIM // H          # 64
NFT = FDIM // 128       # 4 fo-tiles
NCH = T // 128          # 16 t2-chunks
QB = 2                  # q blocks
QBS = T // QB           # 1024
N_CORES = 8

BF16 = ml_dtypes.bfloat16

_cache = {}


def _build_nc():
    import concourse.bass as bass
    import concourse.mybir as mybir
    from concourse import bacc, tile

    f32 = mybir.dt.float32
    bf16 = mybir.dt.bfloat16
    Exp = mybir.ActivationFunctionType.Exp
    Alu = mybir.AluOpType

    nc = bacc.Bacc("TRN2", target_bir_lowering=False, debug=False,
                   num_devices=N_CORES)

    # DRAM I/O (per-core shard shapes)
    xqT = nc.dram_tensor("xqT", [FDIM, T], bf16, kind="ExternalInput")
    xkT = nc.dram_tensor("xkT", [FDIM, T], bf16, kind="ExternalInput")
    xvT = nc.dram_tensor("xvT", [FDIM, T], bf16, kind="ExternalInput")
    wqT = nc.dram_tensor("wqT", [FDIM, FDIM], bf16, kind="ExternalInput")
    wkT = nc.dram_tensor("wkT", [FDIM, FDIM], bf16, kind="ExternalInput")
    wvT = nc.dram_tensor("wvT", [FDIM, FDIM], bf16, kind="ExternalInput")
    woT = nc.dram_tensor("woT", [FDIM, FDIM], bf16, kind="ExternalInput")
    bq = nc.dram_tensor("bq", [FDIM], f32, kind="ExternalInput")
    bk = nc.dram_tensor("bk", [FDIM], f32, kind="ExternalInput")
    bv = nc.dram_tensor("bv", [FDIM], f32, kind="ExternalInput")
    bo = nc.dram_tensor("bo", [FDIM], f32, kind="ExternalInput")
    mbar = nc.dram_tensor("mbar", [NCH, 128, T], bf16, kind="ExternalInput")
    yT = nc.dram_tensor("yT", [FDIM, T], f32, kind="ExternalOutput")
    # DRAM bounce rows for partition-broadcasting softmax reciprocals
    rscratch = nc.dram_tensor("rscratch", [QB * H, QBS], f32)

    import os
    dbg = os.environ.get("KERNEL_DEBUG_DUMPS") == "1"
    if dbg:
        dbg_qt = nc.dram_tensor("dbg_qt", [128, T], bf16, kind="ExternalOutput")
        dbg_kp = nc.dram_tensor("dbg_kp", [2, 128, T], bf16, kind="ExternalOutput")
        dbg_va = nc.dram_tensor("dbg_va", [128, H * (DK + 1)], bf16, kind="ExternalOutput")
        dbg_pm = nc.dram_tensor("dbg_pm", [128, QBS], bf16, kind="ExternalOutput")
        dbg_osb = nc.dram_tensor("dbg_osb", [64, QBS], bf16, kind="ExternalOutput")
        dbg_rb = nc.dram_tensor("dbg_rb", [2, QBS], f32, kind="ExternalOutput")

    with tile.TileContext(nc) as tc:
        with (
            tc.tile_pool(name="consts", bufs=1) as consts,
            tc.tile_pool(name="qt", bufs=1) as qt_pool,
            tc.tile_pool(name="kpad", bufs=1) as kpad_pool,
            tc.tile_pool(name="vaug", bufs=1) as vaug_pool,
            tc.tile_pool(name="osb", bufs=1) as osb_pool,
            tc.tile_pool(name="ysb", bufs=2) as ysb_pool,
        ):
            # ---- consts: weights + biases ----
            wq_sb = [consts.tile([128, FDIM], bf16, tag=f"wq{fc}", name=f"wq{fc}") for fc in range(4)]
            wk_sb = [consts.tile([128, FDIM], bf16, tag=f"wk{fc}", name=f"wk{fc}") for fc in range(4)]
            wv_sb = [consts.tile([128, FDIM], bf16, tag=f"wv{fc}", name=f"wv{fc}") for fc in range(4)]
            wo_sb = [consts.tile([64, FDIM], bf16, tag=f"wo{h}", name=f"wo{h}") for h in range(H)]
            for fc in range(4):
                nc.sync.dma_start(out=wq_sb[fc][:], in_=wqT[fc * 128:(fc + 1) * 128, :])
                nc.sync.dma_start(out=wk_sb[fc][:], in_=wkT[fc * 128:(fc + 1) * 128, :])
                nc.sync.dma_start(out=wv_sb[fc][:], in_=wvT[fc * 128:(fc + 1) * 128, :])
            for h in range(H):
                nc.sync.dma_start(out=wo_sb[h][:], in_=woT[h * 64:(h + 1) * 64, :])

            bq_sb = consts.tile([128, NFT], f32, tag="bq", name="bq")
            bk_sb = consts.tile([128, NFT], f32, tag="bk", name="bk")
            bo_sb = consts.tile([128, NFT], f32, tag="bo", name="bo")
            for b_dram, b_t in ((bq, bq_sb), (bk, bk_sb), (bo, bo_sb)):
                nc.sync.dma_start(out=b_t[:], in_=b_dram.ap().rearrange("(j p) -> p j", p=128))
            bv_bcast = consts.tile([128, FDIM], f32, tag="bv_bcast", name="bv_bcast")
            nc.sync.dma_start(
                out=bv_bcast[:],
                in_=bv.ap().rearrange("(a f) -> a f", a=1).to_broadcast([128, FDIM]))

            # ---- persistent activation tiles ----
            qT_sb = [qt_pool.tile([128, T], bf16, tag=f"qT{j}", name=f"qT{j}") for j in range(NFT)]
            kT_sb = [kpad_pool.tile([128, T], bf16, tag=f"kT{j}", name=f"kT{j}") for j in range(NFT)]
            vaug = [vaug_pool.tile([128, H * (DK + 1)], bf16, tag=f"va{tt}", name=f"va{tt}")
                    for tt in range(NCH)]
            # ones column per head in V_aug
            for tt in range(NCH):
                va = vaug[tt][:].rearrange("p (h d) -> p h d", d=DK + 1)
                nc.vector.memset(va[:, :, DK:DK + 1], 1.0)

            o_sb = {}
            for qb in range(QB):
                for h in range(H):
                    o_sb[(qb, h)] = osb_pool.tile([64, QBS], bf16, tag="osb",
                                                  bufs=10, name=f"o{qb}_{h}")

            # ============ PSUM pool (8 banks total, shared by phases) ======
            # tag "s":  2 x [128,1024] f32 = 4 banks  (scores / QK-proj)
            # tag o0/o1: 1 x [128,1024]-sized slot each = 4 banks
            #            (attnV accumulators, V-proj, P3 y-tiles)
            with (
                tc.tile_pool(name="xt", bufs=5) as xt_pool,
                tc.tile_pool(name="mask", bufs=16) as mask_pool,
                tc.tile_pool(name="praw", bufs=2) as praw_pool,
                tc.tile_pool(name="pm", bufs=2) as pm_pool,
                tc.tile_pool(name="rb", bufs=2) as rb_pool,
                tc.tile_pool(name="psum", bufs=2, space="PSUM") as psum_pool,
            ):
                def load_xT(xT_dram, tag):
                    tiles = []
                    for fc in range(4):
                        xt = xt_pool.tile([128, T], bf16, tag=tag, bufs=4, name="xt")
                        nc.sync.dma_start(out=xt[:], in_=xT_dram[fc * 128:(fc + 1) * 128, :])
                        tiles.append(xt)
                    return tiles

                def v_proj(xts):
                    for tt in range(NCH):
                        ps = psum_pool.tile([128, 512], mybir.dt.float32,
                                            tag=f"o{tt % 2}", bufs=1, name="vp")
                        for fc in range(4):
                            nc.tensor.matmul(
                                ps[:],
                                xts[fc][:, tt * 128:(tt + 1) * 128],
                                wv_sb[fc][:],
                                start=(fc == 0), stop=(fc == 3),
                            )
                        va = vaug[tt][:].rearrange("p (h d) -> p h d", d=DK + 1)
                        nc.vector.scalar_tensor_tensor(
                            out=va[:, :, 0:DK],
                            in0=ps[:].rearrange("p (h d) -> p h d", d=DK),
                            scalar=1.0,
                            in1=bv_bcast[:].rearrange("p (h d) -> p h d", d=DK),
                            op0=Alu.mult, op1=Alu.add,
                        )

                def qk_proj(j, xts_q, xts_k):
                    for xts, w_sb, b_t, dst in ((xts_q, wq_sb, bq_sb, qT_sb[j]),
                                                (xts_k, wk_sb, bk_sb, kT_sb[j])):
                        for s in range(4):
                            ps = psum_pool.tile([128, 512], mybir.dt.float32,
                                                tag="s", name="qkp")
                            for fc in range(4):
                                nc.tensor.matmul(
                                    ps[:],
                                    w_sb[fc][:, j * 128:(j + 1) * 128],
                                    xts[fc][:, s * 512:(s + 1) * 512],
                                    start=(fc == 0), stop=(fc == 3),
                                )
                            nc.vector.tensor_scalar_add(
                                dst[:, s * 512:(s + 1) * 512], ps[:], b_t[:, j:j + 1])

                def p3(qb):
                    qsl = slice(qb * QBS, (qb + 1) * QBS)
                    for i in range(NFT):
                        y_ps = psum_pool.tile([128, QBS], mybir.dt.float32,
                                              tag=f"o{i % 2}", bufs=1, name="y")
                        for s in range(2):
                            for h in range(H):
                                nc.tensor.matmul(
                                    y_ps[:, s * 512:(s + 1) * 512],
                                    wo_sb[h][:, i * 128:(i + 1) * 128],
                                    o_sb[(qb, h)][:, s * 512:(s + 1) * 512],
                                    start=(h == 0), stop=(h == H - 1),
                                )
                        y_sb = ysb_pool.tile([128, QBS], mybir.dt.float32, tag="ysb", name="ysb")
                        nc.vector.tensor_scalar_add(y_sb[:], y_ps[:], bo_sb[:, i:i + 1])
                        nc.sync.dma_start(out=yT[i * 128:(i + 1) * 128, qsl], in_=y_sb[:])

                # ---- P1 prefix: V first (every head needs it), then QK j=0
                xts_v = load_xT(xvT, "xq")  # slots reused by xq after v_proj
                v_proj(xts_v)
                xts_q = load_xT(xqT, "xq")
                xts_k = load_xT(xkT, "xk")
                qk_proj(0, xts_q, xts_k)

                if dbg:
                    nc.sync.dma_start(out=dbg_qt.ap(), in_=qT_sb[0][:])
                    nc.sync.dma_start(out=dbg_kp.ap()[0], in_=kT_sb[0][:])
                    nc.sync.dma_start(out=dbg_kp.ap()[1], in_=kT_sb[1][:])
                    nc.sync.dma_start(out=dbg_va.ap(), in_=vaug[0][:])

                # ---- P2 + P3, with remaining QK projections interleaved ----
                for qb in range(QB):
                    qsl = slice(qb * QBS, (qb + 1) * QBS)
                    mask_t = []
                    for c in range(NCH):
                        mt = mask_pool.tile([128, QBS], bf16, tag="mask", name="mask")
                        nc.sync.dma_start(out=mt[:], in_=mbar[c, :, qsl])
                        mask_t.append(mt)

                    # head pairs: even head on partitions 0-63, odd on 64-127
                    # of fo-tile j -> their score matmuls use independent
                    # (64,128) array tiles @row-0 / @row-64 and overlap.
                    for hp in range(H // 2):
                        j = hp
                        heads = (2 * hp, 2 * hp + 1)
                        o_ps = {}
                        for hi, h in enumerate(heads):
                            o_ps[h] = psum_pool.tile([DK + 1, QBS], mybir.dt.float32,
                                                     tag=f"o{hi}", bufs=1, name=f"o{hi}")
                        for c in range(NCH):
                            s_ps = {}
                            for hi, h in enumerate(heads):
                                s_ps[h] = psum_pool.tile([128, QBS], mybir.dt.float32,
                                                         tag="s", name="s")
                            for s in range(2):
                                for hi, h in enumerate(heads):
                                    pr = slice(hi * 64, hi * 64 + 64)
                                    nc.tensor.matmul(
                                        s_ps[h][:, s * 512:(s + 1) * 512],
                                        kT_sb[j][pr, c * 128:(c + 1) * 128],
                                        qT_sb[j][pr, qb * QBS + s * 512: qb * QBS + (s + 1) * 512],
                                        start=True, stop=True,
                                    )
                            for hi, h in enumerate(heads):
                                p_raw = praw_pool.tile([128, QBS], bf16, tag="praw", name="praw")
                                nc.scalar.activation(p_raw[:], s_ps[h][:], Exp,
                                                     bias=0.0, scale=0.125)
                                p_m = pm_pool.tile([128, QBS], bf16, tag="pm", name="pm")
                                nc.vector.tensor_mul(p_m[:], p_raw[:], mask_t[c][:])
                                if dbg and qb == 0 and h == 0 and c == 0:
                                    nc.sync.dma_start(out=dbg_pm.ap(), in_=p_m[:])
                                for s in range(2):
                                    nc.tensor.matmul(
                                        o_ps[h][:, s * 512:(s + 1) * 512],
                                        vaug[c][:, h * (DK + 1):(h + 1) * (DK + 1)],
                                        p_m[:, s * 512:(s + 1) * 512],
                                        start=(c == 0), stop=(c == NCH - 1),
                                    )
                        # epilogue: divide by the denominator (row DK of o_ps).
                        # reciprocal is ~8 cyc/elem/lane -> split the [1,1024]
                        # row over 8 partitions via SBUF->SBUF DMA; a DRAM
                        # bounce row broadcasts it across partitions 0-63.
                        for h in heads:
                            rb = rb_pool.tile([128, QBS], mybir.dt.float32, tag="rb", name="rb")
                            rbs = rb_pool.tile([8, QBS // 8], mybir.dt.float32, tag="rbs", name="rbs")
                            rbr = rb_pool.tile([8, QBS // 8], mybir.dt.float32, tag="rbr", name="rbr")
                            nc.vector.tensor_copy(rb[64:65, :], o_ps[h][DK:DK + 1, :])
                            nc.sync.dma_start(out=rbs[:], in_=rb[64:65, :])
                            nc.vector.reciprocal(rbr[:], rbs[:])
                            rrow = rscratch.ap()[qb * H + h: qb * H + h + 1, :]
                            nc.sync.dma_start(out=rrow, in_=rbr[:])
                            nc.sync.dma_start(out=rb[0:64, :],
                                              in_=rrow.to_broadcast([64, QBS]))
                            nc.vector.tensor_mul(o_sb[(qb, h)][:], o_ps[h][0:DK, :],
                                                 rb[0:64, :])
                            if dbg and qb == 0 and h == 0:
                                nc.sync.dma_start(out=dbg_rb.ap()[0:1, :], in_=rb[0:1, :])
                                nc.sync.dma_start(out=dbg_rb.ap()[1:2, :], in_=rb[64:65, :])
                                nc.sync.dma_start(out=dbg_osb.ap(), in_=o_sb[(qb, h)][:])

                        # overlap the next fo-tile's Q/K projection with the
                        # next pair's attention (first qb only)
                        if qb == 0 and hp + 1 < NFT:
                            qk_proj(hp + 1, xts_q, xts_k)

                    p3(qb)

    nc.compile()
    return nc


def _get_nc():
    if "nc" not in _cache:
        _cache["nc"] = _build_nc()
    return _cache["nc"]


def _make_in_maps(inputs):
    query = np.asarray(inputs["query"], np.float32)
    key = np.asarray(inputs["key"], np.float32)
    value = np.asarray(inputs["value"], np.float32)
    mask = np.asarray(inputs["mask"], bool)
    shared = {
        "wqT": np.ascontiguousarray(np.asarray(inputs["Wq"], np.float32).T).astype(BF16),
        "wkT": np.ascontiguousarray(np.asarray(inputs["Wk"], np.float32).T).astype(BF16),
        "wvT": np.ascontiguousarray(np.asarray(inputs["Wv"], np.float32).T).astype(BF16),
        "woT": np.ascontiguousarray(np.asarray(inputs["Wo"], np.float32).T).astype(BF16),
        "bq": np.asarray(inputs["bq"], np.float32),
        "bk": np.asarray(inputs["bk"], np.float32),
        "bv": np.asarray(inputs["bv"], np.float32),
        "bo": np.asarray(inputs["bo"], np.float32),
    }
    in_maps = []
    for b in range(N_CORES):
        m = dict(shared)
        m["xqT"] = np.ascontiguousarray(query[b].T).astype(BF16)
        m["xkT"] = np.ascontiguousarray(key[b].T).astype(BF16)
        m["xvT"] = np.ascontiguousarray(value[b].T).astype(BF16)
        mb = (~mask[b]).T.astype(BF16)          # (1 - mask)^T, [t2, q]
        m["mbar"] = np.ascontiguousarray(mb.reshape(NCH, 128, T))
        in_maps.append(m)
    return in_maps


def run(inputs, trace=False, **kwargs):
    from concourse.bass_utils import run_bass_kernel_spmd
    nc = _get_nc()
    res = run_bass_kernel_spmd(nc, _make_in_maps(inputs),
                               core_ids=list(range(N_CORES)),
                               trace=trace, **kwargs)
    y = np.stack([np.asarray(res.results[b]["yT"], np.float32).T
                  for b in range(N_CORES)])
    return y, res


def kernel(**inputs) -> np.ndarray:
    y, _ = run(inputs, trace=False)
    return y


# revision 24
# speedup vs baseline: 1.7607x; 1.2543x over previous
"""Multi-head attention (B=8, T=2048, D=512, H=8) on 8 TRN2 NeuronCores.

Sharding: data-parallel over batch — one batch element per core, no
collectives. Host-side prep (part of shard/unshard): transpose x inputs to
[D, T], cast matmul operands to bf16, pass (1 - mask)^T chunk-major, and
transpose the per-core output y^T back to [T, D].

Per-core algorithm ("transposed flash", everything in one PE tiling mode):
  P1: Q^T = Wq x^T, K^T = Wk x^T (padded per-head into zero-padded 128-row
      tiles), V = x Wv^T (augmented with a ones column per head for the
      softmax denominator).
  P2: per (q-block, head, t2-chunk):
        S^T[t2,q] = Kpad_h^T.T @ Q^T          (PSUM, scale deferred)
        P_raw     = exp(S^T / 8)              (ScalarE, PSUM -> SBUF bf16)
        P         = P_raw * (1-mask)^T        (VectorE; equals reference's
                                               where(mask,-inf) + where(mask,0)
                                               since exp(-1e4) == 0 in f32)
        O_aug^T  += Vaug_h.T @ P              (PSUM accum; row 64 = denom)
      epilogue: recip(denom) -> broadcast -> O^T = O_aug^T[0:64] * recip.
  P3: y^T = Wo^T.T @ O^T (+bo), DMA out.

Biases bq, bk, bo are applied (per-partition fused adds); bv via a
broadcast add on V eviction. No max-subtraction in softmax: scores are
O(6) so exp is safe in f32, matching the reference to ~bf16 accuracy.
"""

import numpy as np
import ml_dtypes

B, T, FDIM, H = 8, 2048, 512, 8
DK = FDIM // H          # 64
NFT = FDIM // 128       # 4 fo-tiles
NCH = T // 128          # 16 t2-chunks
QB = 2                  # q blocks
QBS = T // QB           # 1024
N_CORES = 8

BF16 = ml_dtypes.bfloat16

_cache = {}


def _build_nc():
    import concourse.bass as bass
    import concourse.mybir as mybir
    from concourse import bacc, tile

    f32 = mybir.dt.float32
    bf16 = mybir.dt.bfloat16
    Exp = mybir.ActivationFunctionType.Exp
    Alu = mybir.AluOpType

    nc = bacc.Bacc("TRN2", target_bir_lowering=False, debug=False,
                   num_devices=N_CORES)

    # DRAM I/O (per-core shard shapes)
    xqT = nc.dram_tensor("xqT", [FDIM, T], bf16, kind="ExternalInput")
    xkT = nc.dram_tensor("xkT", [FDIM, T], bf16, kind="ExternalInput")
    xvT = nc.dram_tensor("xvT", [FDIM, T], bf16, kind="ExternalInput")
    wqT = nc.dram_tensor("wqT", [FDIM, FDIM], bf16, kind="ExternalInput")
    wkT = nc.dram_tensor("wkT", [FDIM, FDIM], bf16, kind="ExternalInput")
    wvT = nc.dram_tensor("wvT", [FDIM, FDIM], bf16, kind="ExternalInput")
    woT = nc.dram_tensor("woT", [FDIM, FDIM], bf16, kind="ExternalInput")
    bq = nc.dram_tensor("bq", [FDIM], f32, kind="ExternalInput")
    bk = nc.dram_tensor("bk", [FDIM], f32, kind="ExternalInput")
    bv = nc.dram_tensor("bv", [FDIM], f32, kind="ExternalInput")
    bo = nc.dram_tensor("bo", [FDIM], f32, kind="ExternalInput")
    mbar = nc.dram_tensor("mbar", [NCH, 128, T], bf16, kind="ExternalInput")
    yT = nc.dram_tensor("yT", [FDIM, T], f32, kind="ExternalOutput")
    # DRAM bounce rows for partition-broadcasting softmax reciprocals
    rscratch = nc.dram_tensor("rscratch", [QB * H, QBS], f32)

    import os
    dbg = os.environ.get("KERNEL_DEBUG_DUMPS") == "1"
    if dbg:
        dbg_qt = nc.dram_tensor("dbg_qt", [128, T], bf16, kind="ExternalOutput")
        dbg_kp = nc.dram_tensor("dbg_kp", [2, 128, T], bf16, kind="ExternalOutput")
        dbg_va = nc.dram_tensor("dbg_va", [128, H * (DK + 1)], bf16, kind="ExternalOutput")
        dbg_pm = nc.dram_tensor("dbg_pm", [128, QBS], bf16, kind="ExternalOutput")
        dbg_osb = nc.dram_tensor("dbg_osb", [64, QBS], bf16, kind="ExternalOutput")
        dbg_rb = nc.dram_tensor("dbg_rb", [2, QBS], f32, kind="ExternalOutput")

    with tile.TileContext(nc) as tc:
        with (
            tc.tile_pool(name="consts", bufs=1) as consts,
            tc.tile_pool(name="qt", bufs=1) as qt_pool,
            tc.tile_pool(name="kpad", bufs=1) as kpad_pool,
            tc.tile_pool(name="vaug", bufs=1) as vaug_pool,
            tc.tile_pool(name="osb", bufs=1) as osb_pool,
            tc.tile_pool(name="ysb", bufs=2) as ysb_pool,
        ):
            # ---- consts: weights + biases ----
            wq_sb = [consts.tile([128, FDIM], bf16, tag=f"wq{fc}", name=f"wq{fc}") for fc in range(4)]
            wk_sb = [consts.tile([128, FDIM], bf16, tag=f"wk{fc}", name=f"wk{fc}") for fc in range(4)]
            wv_sb = [consts.tile([128, FDIM], bf16, tag=f"wv{fc}", name=f"wv{fc}") for fc in range(4)]
            wo_sb = [consts.tile([64, FDIM], bf16, tag=f"wo{h}", name=f"wo{h}") for h in range(H)]
            for fc in range(4):
                nc.sync.dma_start(out=wq_sb[fc][:], in_=wqT[fc * 128:(fc + 1) * 128, :])
                nc.sync.dma_start(out=wk_sb[fc][:], in_=wkT[fc * 128:(fc + 1) * 128, :])
                nc.sync.dma_start(out=wv_sb[fc][:], in_=wvT[fc * 128:(fc + 1) * 128, :])
            for h in range(H):
                nc.sync.dma_start(out=wo_sb[h][:], in_=woT[h * 64:(h + 1) * 64, :])

            bq_sb = consts.tile([128, NFT], f32, tag="bq", name="bq")
            bk_sb = consts.tile([128, NFT], f32, tag="bk", name="bk")
            bo_sb = consts.tile([128, NFT], f32, tag="bo", name="bo")
            for b_dram, b_t in ((bq, bq_sb), (bk, bk_sb), (bo, bo_sb)):
                nc.sync.dma_start(out=b_t[:], in_=b_dram.ap().rearrange("(j p) -> p j", p=128))
            bv_bcast = consts.tile([128, FDIM], f32, tag="bv_bcast", name="bv_bcast")
            nc.sync.dma_start(
                out=bv_bcast[:],
                in_=bv.ap().rearrange("(a f) -> a f", a=1).to_broadcast([128, FDIM]))

            # ---- persistent activation tiles ----
            qT_sb = [qt_pool.tile([128, T], bf16, tag=f"qT{j}", name=f"qT{j}") for j in range(NFT)]
            kpad = [kpad_pool.tile([128, T], bf16, tag=f"kp{h}", name=f"kp{h}") for h in range(H)]
            # zero the unused head-half of each padded K tile, once
            for h in range(H):
                half = slice(64, 128) if h % 2 == 0 else slice(0, 64)
                nc.vector.memset(kpad[h][half, :], 0.0)
            vaug = [vaug_pool.tile([128, H * (DK + 1)], bf16, tag=f"va{tt}", name=f"va{tt}")
                    for tt in range(NCH)]
            # ones column per head in V_aug
            for tt in range(NCH):
                va = vaug[tt][:].rearrange("p (h d) -> p h d", d=DK + 1)
                nc.vector.memset(va[:, :, DK:DK + 1], 1.0)

            o_sb = {}
            for qb in range(QB):
                for h in range(H):
                    o_sb[(qb, h)] = osb_pool.tile([64, QBS], bf16, tag="osb",
                                                  bufs=10, name=f"o{qb}_{h}")

            # ============ PSUM pool (8 banks total, shared by phases) ======
            # tag "s":  2 x [128,1024] f32 = 4 banks  (scores / QK-proj)
            # tag o0/o1: 1 x [128,1024]-sized slot each = 4 banks
            #            (attnV accumulators, V-proj, P3 y-tiles)
            with (
                tc.tile_pool(name="xt", bufs=5) as xt_pool,
                tc.tile_pool(name="mask", bufs=16) as mask_pool,
                tc.tile_pool(name="praw", bufs=2) as praw_pool,
                tc.tile_pool(name="pm", bufs=2) as pm_pool,
                tc.tile_pool(name="rb", bufs=1) as rb_pool,
                tc.tile_pool(name="psum", bufs=2, space="PSUM") as psum_pool,
            ):
                def load_xT(xT_dram, tag):
                    tiles = []
                    for fc in range(4):
                        xt = xt_pool.tile([128, T], bf16, tag=tag, bufs=4, name="xt")
                        nc.sync.dma_start(out=xt[:], in_=xT_dram[fc * 128:(fc + 1) * 128, :])
                        tiles.append(xt)
                    return tiles

                def v_proj(xts):
                    for tt in range(NCH):
                        ps = psum_pool.tile([128, 512], mybir.dt.float32,
                                            tag=f"o{tt % 2}", bufs=1, name="vp")
                        for fc in range(4):
                            nc.tensor.matmul(
                                ps[:],
                                xts[fc][:, tt * 128:(tt + 1) * 128],
                                wv_sb[fc][:],
                                start=(fc == 0), stop=(fc == 3),
                            )
                        va = vaug[tt][:].rearrange("p (h d) -> p h d", d=DK + 1)
                        nc.vector.scalar_tensor_tensor(
                            out=va[:, :, 0:DK],
                            in0=ps[:].rearrange("p (h d) -> p h d", d=DK),
                            scalar=1.0,
                            in1=bv_bcast[:].rearrange("p (h d) -> p h d", d=DK),
                            op0=Alu.mult, op1=Alu.add,
                        )

                def qk_proj(j, xts_q, xts_k):
                    for xts, w_sb, b_t, dst in ((xts_q, wq_sb, bq_sb, qT_sb[j]),
                                                (xts_k, wk_sb, bk_sb, None)):
                        for s in range(4):
                            ps = psum_pool.tile([128, 512], mybir.dt.float32,
                                                tag="s", name="qkp")
                            for fc in range(4):
                                nc.tensor.matmul(
                                    ps[:],
                                    w_sb[fc][:, j * 128:(j + 1) * 128],
                                    xts[fc][:, s * 512:(s + 1) * 512],
                                    start=(fc == 0), stop=(fc == 3),
                                )
                            sl = slice(s * 512, (s + 1) * 512)
                            if dst is not None:
                                nc.vector.tensor_scalar_add(dst[:, sl], ps[:],
                                                            b_t[:, j:j + 1])
                            else:  # K: evict into the two padded per-head tiles
                                nc.vector.tensor_scalar_add(
                                    kpad[2 * j][0:64, sl], ps[0:64, :],
                                    b_t[0:64, j:j + 1])
                                nc.vector.tensor_scalar_add(
                                    kpad[2 * j + 1][64:128, sl], ps[64:128, :],
                                    b_t[64:128, j:j + 1])

                def p3(qb):
                    qsl = slice(qb * QBS, (qb + 1) * QBS)
                    for i in range(NFT):
                        y_ps = psum_pool.tile([128, QBS], mybir.dt.float32,
                                              tag=f"o{i % 2}", bufs=1, name="y")
                        for s in range(2):
                            for h in range(H):
                                nc.tensor.matmul(
                                    y_ps[:, s * 512:(s + 1) * 512],
                                    wo_sb[h][:, i * 128:(i + 1) * 128],
                                    o_sb[(qb, h)][:, s * 512:(s + 1) * 512],
                                    start=(h == 0), stop=(h == H - 1),
                                )
                        y_sb = ysb_pool.tile([128, QBS], mybir.dt.float32, tag="ysb", name="ysb")
                        nc.vector.tensor_scalar_add(y_sb[:], y_ps[:], bo_sb[:, i:i + 1])
                        nc.sync.dma_start(out=yT[i * 128:(i + 1) * 128, qsl], in_=y_sb[:])

                # ---- P1 prefix: V first (every head needs it), then QK j=0
                xts_v = load_xT(xvT, "xq")  # slots reused by xq after v_proj
                v_proj(xts_v)
                xts_q = load_xT(xqT, "xq")
                xts_k = load_xT(xkT, "xk")
                qk_proj(0, xts_q, xts_k)

                if dbg:
                    nc.sync.dma_start(out=dbg_qt.ap(), in_=qT_sb[0][:])
                    nc.sync.dma_start(out=dbg_kp.ap()[0], in_=kpad[0][:])
                    nc.sync.dma_start(out=dbg_kp.ap()[1], in_=kpad[1][:])
                    nc.sync.dma_start(out=dbg_va.ap(), in_=vaug[0][:])

                # ---- P2 + P3, with remaining QK projections interleaved ----
                for qb in range(QB):
                    qsl = slice(qb * QBS, (qb + 1) * QBS)
                    mask_t = []
                    for c in range(NCH):
                        mt = mask_pool.tile([128, QBS], bf16, tag="mask", name="mask")
                        nc.sync.dma_start(out=mt[:], in_=mbar[c, :, qsl])
                        mask_t.append(mt)

                    for h in range(H):
                        j = h // 2
                        o_ps = psum_pool.tile([DK + 1, QBS], mybir.dt.float32,
                                              tag=f"o{h % 2}", bufs=1, name="o")
                        for c in range(NCH):
                            s_ps = psum_pool.tile([128, QBS], mybir.dt.float32,
                                                  tag="s", name="s")
                            for s in range(2):
                                nc.tensor.matmul(
                                    s_ps[:, s * 512:(s + 1) * 512],
                                    kpad[h][:, c * 128:(c + 1) * 128],
                                    qT_sb[j][:, qb * QBS + s * 512: qb * QBS + (s + 1) * 512],
                                    start=True, stop=True,
                                )
                            p_raw = praw_pool.tile([128, QBS], bf16, tag="praw", name="praw")
                            nc.scalar.activation(p_raw[:], s_ps[:], Exp,
                                                 bias=0.0, scale=0.125)
                            p_m = pm_pool.tile([128, QBS], bf16, tag="pm", name="pm")
                            nc.vector.tensor_mul(p_m[:], p_raw[:], mask_t[c][:])
                            if dbg and qb == 0 and h == 0 and c == 0:
                                nc.sync.dma_start(out=dbg_pm.ap(), in_=p_m[:])
                            for s in range(2):
                                nc.tensor.matmul(
                                    o_ps[:, s * 512:(s + 1) * 512],
                                    vaug[c][:, h * (DK + 1):(h + 1) * (DK + 1)],
                                    p_m[:, s * 512:(s + 1) * 512],
                                    start=(c == 0), stop=(c == NCH - 1),
                                )
                        # epilogue: divide by the denominator (row DK of o_ps).
                        # reciprocal is ~8 cyc/elem/lane -> split the [1,1024]
                        # row over 8 partitions via SBUF->SBUF DMA; a DRAM
                        # bounce row broadcasts it across partitions 0-63.
                        rb = rb_pool.tile([128, QBS], mybir.dt.float32, tag="rb", name="rb")
                        rbs = rb_pool.tile([8, QBS // 8], mybir.dt.float32, tag="rbs", name="rbs")
                        rbr = rb_pool.tile([8, QBS // 8], mybir.dt.float32, tag="rbr", name="rbr")
                        nc.vector.tensor_copy(rb[64:65, :], o_ps[DK:DK + 1, :])
                        nc.sync.dma_start(out=rbs[:], in_=rb[64:65, :])
                        nc.vector.reciprocal(rbr[:], rbs[:])
                        rrow = rscratch.ap()[qb * H + h: qb * H + h + 1, :]
                        nc.sync.dma_start(out=rrow, in_=rbr[:])
                        nc.sync.dma_start(out=rb[0:64, :],
                                          in_=rrow.to_broadcast([64, QBS]))
                        nc.vector.tensor_mul(o_sb[(qb, h)][:], o_ps[0:DK, :],
                                             rb[0:64, :])
                        if dbg and qb == 0 and h == 0:
                            nc.sync.dma_start(out=dbg_rb.ap()[0:1, :], in_=rb[0:1, :])
                            nc.sync.dma_start(out=dbg_rb.ap()[1:2, :], in_=rb[64:65, :])
                            nc.sync.dma_start(out=dbg_osb.ap(), in_=o_sb[(qb, h)][:])

                        # overlap the next fo-tile's Q/K projection with the
                        # next heads' attention (first qb only)
                        if qb == 0 and h % 2 == 1 and h // 2 + 1 < NFT:
                            qk_proj(h // 2 + 1, xts_q, xts_k)

                    p3(qb)

    nc.compile()
    return nc


def _get_nc():
    if "nc" not in _cache:
        _cache["nc"] = _build_nc()
    return _cache["nc"]


def _make_in_maps(inputs):
    query = np.asarray(inputs["query"], np.float32)
    key = np.asarray(inputs["key"], np.float32)
    value = np.asarray(inputs["value"], np.float32)
    mask = np.asarray(inputs["mask"], bool)
    shared = {
        "wqT": np.ascontiguousarray(np.asarray(inputs["Wq"], np.float32).T).astype(BF16),
        "wkT": np.ascontiguousarray(np.asarray(inputs["Wk"], np.float32).T).astype(BF16),
        "wvT": np.ascontiguousarray(np.asarray(inputs["Wv"], np.float32).T).astype(BF16),
        "woT": np.ascontiguousarray(np.asarray(inputs["Wo"], np.float32).T).astype(BF16),
        "bq": np.asarray(inputs["bq"], np.float32),
        "bk": np.asarray(inputs["bk"], np.float32),
        "bv": np.asarray(inputs["bv"], np.float32),
        "bo": np.asarray(inputs["bo"], np.float32),
    }
    in_maps = []
    for b in range(N_CORES):
        m = dict(shared)
        m["xqT"] = np.ascontiguousarray(query[b].T).astype(BF16)
        m["xkT"] = np.ascontiguousarray(key[b].T).astype(BF16)
        m["xvT"] = np.ascontiguousarray(value[b].T).astype(BF16)
        mb = (~mask[b]).T.astype(BF16)          # (1 - mask)^T, [t2, q]
        m["mbar"] = np.ascontiguousarray(mb.reshape(NCH, 128, T))
        in_maps.append(m)
    return in_maps


def run(inputs, trace=False, **kwargs):
    from concourse.bass_utils import run_bass_kernel_spmd
    nc = _get_nc()
    res = run_bass_kernel_spmd(nc, _make_in_maps(inputs),
                               core_ids=list(range(N_CORES)),
                               trace=trace, **kwargs)
    y = np.stack([np.asarray(res.results[b]["yT"], np.float32).T
                  for b in range(N_CORES)])
    return y, res


def kernel(**inputs) -> np.ndarray:
    y, _ = run(inputs, trace=False)
    return y


# revision 27
# speedup vs baseline: 1.8265x; 1.0374x over previous
"""Multi-head attention (B=8, T=2048, D=512, H=8) on 8 TRN2 NeuronCores.

Sharding: data-parallel over batch — one batch element per core, no
collectives. Host-side prep (part of shard/unshard): transpose x inputs to
[D, T], cast matmul operands to bf16, pass (1 - mask)^T chunk-major, and
transpose the per-core output y^T back to [T, D].

Per-core algorithm ("transposed flash", everything in one PE tiling mode):
  P1: Q^T = Wq x^T, K^T = Wk x^T (padded per-head into zero-padded 128-row
      tiles), V = x Wv^T (augmented with a ones column per head for the
      softmax denominator).
  P2: per (q-block, head, t2-chunk):
        S^T[t2,q] = Kpad_h^T.T @ Q^T          (PSUM, scale deferred)
        P_raw     = exp(S^T / 8)              (ScalarE, PSUM -> SBUF bf16)
        P         = P_raw * (1-mask)^T        (VectorE; equals reference's
                                               where(mask,-inf) + where(mask,0)
                                               since exp(-1e4) == 0 in f32)
        O_aug^T  += Vaug_h.T @ P              (PSUM accum; row 64 = denom)
      epilogue: recip(denom) -> broadcast -> O^T = O_aug^T[0:64] * recip.
  P3: y^T = Wo^T.T @ O^T (+bo), DMA out.

Biases bq, bk, bo are applied (per-partition fused adds); bv via a
broadcast add on V eviction. No max-subtraction in softmax: scores are
O(6) so exp is safe in f32, matching the reference to ~bf16 accuracy.
"""

import numpy as np
import ml_dtypes

B, T, FDIM, H = 8, 2048, 512, 8
DK = FDIM // H          # 64
NFT = FDIM // 128       # 4 fo-tiles
NCH = T // 128          # 16 t2-chunks
QB = 2                  # q blocks
QBS = T // QB           # 1024
N_CORES = 8

BF16 = ml_dtypes.bfloat16

_cache = {}


def _build_nc():
    import concourse.bass as bass
    import concourse.mybir as mybir
    from concourse import bacc, tile

    f32 = mybir.dt.float32
    bf16 = mybir.dt.bfloat16
    Exp = mybir.ActivationFunctionType.Exp
    Alu = mybir.AluOpType

    nc = bacc.Bacc("TRN2", target_bir_lowering=False, debug=False,
                   num_devices=N_CORES)

    # DRAM I/O (per-core shard shapes)
    xqT = nc.dram_tensor("xqT", [FDIM, T], bf16, kind="ExternalInput")
    xkT = nc.dram_tensor("xkT", [FDIM, T], bf16, kind="ExternalInput")
    xvT = nc.dram_tensor("xvT", [FDIM, T], bf16, kind="ExternalInput")
    wqT = nc.dram_tensor("wqT", [FDIM, FDIM], bf16, kind="ExternalInput")
    wkT = nc.dram_tensor("wkT", [FDIM, FDIM], bf16, kind="ExternalInput")
    wvT = nc.dram_tensor("wvT", [FDIM, FDIM], bf16, kind="ExternalInput")
    woT = nc.dram_tensor("woT", [FDIM, FDIM], bf16, kind="ExternalInput")
    bq = nc.dram_tensor("bq", [FDIM], f32, kind="ExternalInput")
    bk = nc.dram_tensor("bk", [FDIM], f32, kind="ExternalInput")
    bv = nc.dram_tensor("bv", [FDIM], f32, kind="ExternalInput")
    bo = nc.dram_tensor("bo", [FDIM], f32, kind="ExternalInput")
    mbar = nc.dram_tensor("mbar", [NCH, 128, T], bf16, kind="ExternalInput")
    yT = nc.dram_tensor("yT", [FDIM, T], f32, kind="ExternalOutput")
    # DRAM bounce rows for partition-broadcasting softmax reciprocals
    rscratch = nc.dram_tensor("rscratch", [QB * H, QBS], f32)

    import os
    dbg = os.environ.get("KERNEL_DEBUG_DUMPS") == "1"
    if dbg:
        dbg_qt = nc.dram_tensor("dbg_qt", [128, T], bf16, kind="ExternalOutput")
        dbg_kp = nc.dram_tensor("dbg_kp", [2, 128, T], bf16, kind="ExternalOutput")
        dbg_va = nc.dram_tensor("dbg_va", [128, H * (DK + 1)], bf16, kind="ExternalOutput")
        dbg_pm = nc.dram_tensor("dbg_pm", [128, QBS], bf16, kind="ExternalOutput")
        dbg_osb = nc.dram_tensor("dbg_osb", [64, QBS], bf16, kind="ExternalOutput")
        dbg_rb = nc.dram_tensor("dbg_rb", [2, QBS], f32, kind="ExternalOutput")

    with tile.TileContext(nc) as tc:
        with (
            tc.tile_pool(name="consts", bufs=1) as consts,
            tc.tile_pool(name="qt", bufs=1) as qt_pool,
            tc.tile_pool(name="kpad", bufs=1) as kpad_pool,
            tc.tile_pool(name="vaug", bufs=1) as vaug_pool,
            tc.tile_pool(name="osb", bufs=1) as osb_pool,
            tc.tile_pool(name="ysb", bufs=1) as ysb_pool,
        ):
            # ---- consts: weights + biases ----
            wq_sb = [consts.tile([128, FDIM], bf16, tag=f"wq{fc}", name=f"wq{fc}") for fc in range(4)]
            wk_sb = [consts.tile([128, FDIM], bf16, tag=f"wk{fc}", name=f"wk{fc}") for fc in range(4)]
            wv_sb = [consts.tile([128, FDIM], bf16, tag=f"wv{fc}", name=f"wv{fc}") for fc in range(4)]
            wo_sb = [consts.tile([128, FDIM], bf16, tag=f"wo{j}", name=f"wo{j}") for j in range(NFT)]
            for fc in range(4):
                nc.sync.dma_start(out=wq_sb[fc][:], in_=wqT[fc * 128:(fc + 1) * 128, :])
                nc.sync.dma_start(out=wk_sb[fc][:], in_=wkT[fc * 128:(fc + 1) * 128, :])
                nc.sync.dma_start(out=wv_sb[fc][:], in_=wvT[fc * 128:(fc + 1) * 128, :])
            for j in range(NFT):
                nc.sync.dma_start(out=wo_sb[j][:], in_=woT[j * 128:(j + 1) * 128, :])

            bq_sb = consts.tile([128, NFT], f32, tag="bq", name="bq")
            bk_sb = consts.tile([128, NFT], f32, tag="bk", name="bk")
            bo_sb = consts.tile([128, NFT], f32, tag="bo", name="bo")
            for b_dram, b_t in ((bq, bq_sb), (bk, bk_sb), (bo, bo_sb)):
                nc.sync.dma_start(out=b_t[:], in_=b_dram.ap().rearrange("(j p) -> p j", p=128))
            bv_bcast = consts.tile([128, FDIM], f32, tag="bv_bcast", name="bv_bcast")
            nc.sync.dma_start(
                out=bv_bcast[:],
                in_=bv.ap().rearrange("(a f) -> a f", a=1).to_broadcast([128, FDIM]))

            # ---- persistent activation tiles ----
            qT_sb = [qt_pool.tile([128, T], bf16, tag=f"qT{j}", name=f"qT{j}") for j in range(NFT)]
            kpad = [kpad_pool.tile([128, T], bf16, tag=f"kp{h}", name=f"kp{h}") for h in range(H)]
            # zero the unused head-half of each padded K tile, once
            for h in range(H):
                half = slice(64, 128) if h % 2 == 0 else slice(0, 64)
                nc.vector.memset(kpad[h][half, :], 0.0)
            vaug = [vaug_pool.tile([128, H * (DK + 1)], bf16, tag=f"va{tt}", name=f"va{tt}")
                    for tt in range(NCH)]
            # ones column per head in V_aug
            for tt in range(NCH):
                va = vaug[tt][:].rearrange("p (h d) -> p h d", d=DK + 1)
                nc.vector.memset(va[:, :, DK:DK + 1], 1.0)

            o2_sb = {}
            for qb in range(QB):
                for j in range(NFT):
                    o2_sb[(qb, j)] = osb_pool.tile([128, QBS], bf16, tag=f"o2_{qb}_{j}",
                                                   name=f"o2_{qb}_{j}")

            # ============ PSUM pool (8 banks total, shared by phases) ======
            # tag "s":  2 x [128,1024] f32 = 4 banks  (scores / QK-proj)
            # tag o0/o1: 1 x [128,1024]-sized slot each = 4 banks
            #            (attnV accumulators, V-proj, P3 y-tiles)
            with (
                tc.tile_pool(name="xt", bufs=5) as xt_pool,
                tc.tile_pool(name="mask", bufs=16) as mask_pool,
                tc.tile_pool(name="praw", bufs=2) as praw_pool,
                tc.tile_pool(name="pm", bufs=2) as pm_pool,
                tc.tile_pool(name="rb", bufs=1) as rb_pool,
                tc.tile_pool(name="psum", bufs=2, space="PSUM") as psum_pool,
            ):
                def load_xT(xT_dram, tag):
                    tiles = []
                    for fc in range(4):
                        xt = xt_pool.tile([128, T], bf16, tag=tag, bufs=4, name="xt")
                        nc.sync.dma_start(out=xt[:], in_=xT_dram[fc * 128:(fc + 1) * 128, :])
                        tiles.append(xt)
                    return tiles

                def v_proj(xts):
                    for tt in range(NCH):
                        ps = psum_pool.tile([128, 512], mybir.dt.float32,
                                            tag=f"o{tt % 2}", bufs=1, name="vp")
                        for fc in range(4):
                            nc.tensor.matmul(
                                ps[:],
                                xts[fc][:, tt * 128:(tt + 1) * 128],
                                wv_sb[fc][:],
                                start=(fc == 0), stop=(fc == 3),
                            )
                        va = vaug[tt][:].rearrange("p (h d) -> p h d", d=DK + 1)
                        nc.vector.scalar_tensor_tensor(
                            out=va[:, :, 0:DK],
                            in0=ps[:].rearrange("p (h d) -> p h d", d=DK),
                            scalar=1.0,
                            in1=bv_bcast[:].rearrange("p (h d) -> p h d", d=DK),
                            op0=Alu.mult, op1=Alu.add,
                        )

                def proj_groups(j, xts, w_sb, b_t, dst, slices):
                    for s in slices:
                        ps = psum_pool.tile([128, 512], mybir.dt.float32,
                                            tag="s", name="qkp")
                        for fc in range(4):
                            nc.tensor.matmul(
                                ps[:],
                                w_sb[fc][:, j * 128:(j + 1) * 128],
                                xts[fc][:, s * 512:(s + 1) * 512],
                                start=(fc == 0), stop=(fc == 3),
                            )
                        sl = slice(s * 512, (s + 1) * 512)
                        if dst is not None:
                            nc.vector.tensor_scalar_add(dst[:, sl], ps[:],
                                                        b_t[:, j:j + 1])
                        else:  # K: evict into the two padded per-head tiles
                            nc.vector.tensor_scalar_add(
                                kpad[2 * j][0:64, sl], ps[0:64, :],
                                b_t[0:64, j:j + 1])
                            nc.vector.tensor_scalar_add(
                                kpad[2 * j + 1][64:128, sl], ps[64:128, :],
                                b_t[64:128, j:j + 1])

                def q_proj(j, half):
                    proj_groups(j, xts_q, wq_sb, bq_sb, qT_sb[j],
                                range(2 * half, 2 * half + 2))

                def k_proj(j):
                    proj_groups(j, xts_k, wk_sb, bk_sb, None, range(4))

                def p3(qb):
                    qsl = slice(qb * QBS, (qb + 1) * QBS)
                    for i in range(NFT):
                        y_ps = psum_pool.tile([128, QBS], mybir.dt.float32,
                                              tag=f"o{i % 2}", bufs=1, name="y")
                        for s in range(2):
                            for j in range(NFT):
                                nc.tensor.matmul(
                                    y_ps[:, s * 512:(s + 1) * 512],
                                    wo_sb[j][:, i * 128:(i + 1) * 128],
                                    o2_sb[(qb, j)][:, s * 512:(s + 1) * 512],
                                    start=(j == 0), stop=(j == NFT - 1),
                                )
                        y_sb = ysb_pool.tile([128, QBS], mybir.dt.float32, tag="ysb", name="ysb")
                        nc.vector.tensor_scalar_add(y_sb[:], y_ps[:], bo_sb[:, i:i + 1])
                        nc.sync.dma_start(out=yT[i * 128:(i + 1) * 128, qsl], in_=y_sb[:])

                # ---- P1 prefix: V first (every head needs it), then QK j=0
                xts_v = load_xT(xvT, "xq")  # slots reused by xq after v_proj
                v_proj(xts_v)
                xts_q = load_xT(xqT, "xq")
                xts_k = load_xT(xkT, "xk")
                k_proj(0)
                q_proj(0, 0)

                if dbg:
                    nc.sync.dma_start(out=dbg_qt.ap(), in_=qT_sb[0][:])
                    nc.sync.dma_start(out=dbg_kp.ap()[0], in_=kpad[0][:])
                    nc.sync.dma_start(out=dbg_kp.ap()[1], in_=kpad[1][:])
                    nc.sync.dma_start(out=dbg_va.ap(), in_=vaug[0][:])

                # ---- P2 + P3, with remaining QK projections interleaved ----
                for qb in range(QB):
                    qsl = slice(qb * QBS, (qb + 1) * QBS)
                    mask_t = []
                    for c in range(NCH):
                        mt = mask_pool.tile([128, QBS], bf16, tag="mask", name="mask")
                        nc.sync.dma_start(out=mt[:], in_=mbar[c, :, qsl])
                        mask_t.append(mt)

                    for h in range(H):
                        j = h // 2
                        o_ps = psum_pool.tile([DK + 1, QBS], mybir.dt.float32,
                                              tag=f"o{h % 2}", bufs=1, name="o")
                        for c in range(NCH):
                            s_ps = psum_pool.tile([128, QBS], mybir.dt.float32,
                                                  tag="s", name="s")
                            for s in range(2):
                                nc.tensor.matmul(
                                    s_ps[:, s * 512:(s + 1) * 512],
                                    kpad[h][:, c * 128:(c + 1) * 128],
                                    qT_sb[j][:, qb * QBS + s * 512: qb * QBS + (s + 1) * 512],
                                    start=True, stop=True,
                                )
                            p_raw = praw_pool.tile([128, QBS], bf16, tag="praw", name="praw")
                            nc.scalar.activation(p_raw[:], s_ps[:], Exp,
                                                 bias=0.0, scale=0.125)
                            p_m = pm_pool.tile([128, QBS], bf16, tag="pm", name="pm")
                            nc.vector.tensor_mul(p_m[:], p_raw[:], mask_t[c][:])
                            if dbg and qb == 0 and h == 0 and c == 0:
                                nc.sync.dma_start(out=dbg_pm.ap(), in_=p_m[:])
                            for s in range(2):
                                nc.tensor.matmul(
                                    o_ps[:, s * 512:(s + 1) * 512],
                                    vaug[c][:, h * (DK + 1):(h + 1) * (DK + 1)],
                                    p_m[:, s * 512:(s + 1) * 512],
                                    start=(c == 0), stop=(c == NCH - 1),
                                )
                        # epilogue: divide by the denominator (row DK of o_ps).
                        # reciprocal is ~8 cyc/elem/lane -> split the [1,1024]
                        # row over 8 partitions via SBUF->SBUF DMA; a DRAM
                        # bounce row broadcasts it across partitions 0-63.
                        rb = rb_pool.tile([128, QBS], mybir.dt.float32, tag="rb", name="rb")
                        rbs = rb_pool.tile([8, QBS // 8], mybir.dt.float32, tag="rbs", name="rbs")
                        rbr = rb_pool.tile([8, QBS // 8], mybir.dt.float32, tag="rbr", name="rbr")
                        nc.vector.tensor_copy(rb[64:65, :], o_ps[DK:DK + 1, :])
                        nc.sync.dma_start(out=rbs[:], in_=rb[64:65, :])
                        nc.vector.reciprocal(rbr[:], rbs[:])
                        rrow = rscratch.ap()[qb * H + h: qb * H + h + 1, :]
                        nc.sync.dma_start(out=rrow, in_=rbr[:])
                        nc.sync.dma_start(out=rb[0:64, :],
                                          in_=rrow.to_broadcast([64, QBS]))
                        osm = rb_pool.tile([64, QBS], bf16, tag="osm", bufs=3, name="osm")
                        nc.vector.tensor_mul(osm[:], o_ps[0:DK, :], rb[0:64, :])
                        nc.sync.dma_start(
                            out=o2_sb[(qb, h // 2)][(h % 2) * 64:(h % 2) * 64 + 64, :],
                            in_=osm[:])
                        if dbg and qb == 0 and h == 0:
                            nc.sync.dma_start(out=dbg_rb.ap()[0:1, :], in_=rb[0:1, :])
                            nc.sync.dma_start(out=dbg_rb.ap()[1:2, :], in_=rb[64:65, :])
                            nc.sync.dma_start(out=dbg_osb.ap(), in_=osm[:])

                        # overlap remaining projections with the attention
                        # stream: K(j) fully before head 2j; Q(j) per q-block.
                        if qb == 0:
                            step = [("k", 1), ("q", (1, 0)), ("k", 2),
                                    ("q", (2, 0)), ("k", 3), ("q", (3, 0)),
                                    ("q", (0, 1)), None][h]
                        else:
                            step = [("q", (1, 1)), ("q", (2, 1)),
                                    ("q", (3, 1))][h] if h < 3 else None
                        if step is not None:
                            if step[0] == "k":
                                k_proj(step[1])
                            else:
                                q_proj(*step[1])

                    p3(qb)

    nc.compile()
    return nc


def _get_nc():
    if "nc" not in _cache:
        _cache["nc"] = _build_nc()
    return _cache["nc"]


def _make_in_maps(inputs):
    query = np.asarray(inputs["query"], np.float32)
    key = np.asarray(inputs["key"], np.float32)
    value = np.asarray(inputs["value"], np.float32)
    mask = np.asarray(inputs["mask"], bool)
    shared = {
        "wqT": np.ascontiguousarray(np.asarray(inputs["Wq"], np.float32).T).astype(BF16),
        "wkT": np.ascontiguousarray(np.asarray(inputs["Wk"], np.float32).T).astype(BF16),
        "wvT": np.ascontiguousarray(np.asarray(inputs["Wv"], np.float32).T).astype(BF16),
        "woT": np.ascontiguousarray(np.asarray(inputs["Wo"], np.float32).T).astype(BF16),
        "bq": np.asarray(inputs["bq"], np.float32),
        "bk": np.asarray(inputs["bk"], np.float32),
        "bv": np.asarray(inputs["bv"], np.float32),
        "bo": np.asarray(inputs["bo"], np.float32),
    }
    in_maps = []
    for b in range(N_CORES):
        m = dict(shared)
        m["xqT"] = np.ascontiguousarray(query[b].T).astype(BF16)
        m["xkT"] = np.ascontiguousarray(key[b].T).astype(BF16)
        m["xvT"] = np.ascontiguousarray(value[b].T).astype(BF16)
        mb = (~mask[b]).T.astype(BF16)          # (1 - mask)^T, [t2, q]
        m["mbar"] = np.ascontiguousarray(mb.reshape(NCH, 128, T))
        in_maps.append(m)
    return in_maps


def run(inputs, trace=False, **kwargs):
    from concourse.bass_utils import run_bass_kernel_spmd
    nc = _get_nc()
    res = run_bass_kernel_spmd(nc, _make_in_maps(inputs),
                               core_ids=list(range(N_CORES)),
                               trace=trace, **kwargs)
    y = np.stack([np.asarray(res.results[b]["yT"], np.float32).T
                  for b in range(N_CORES)])
    return y, res


def kernel(**inputs) -> np.ndarray:
    y, _ = run(inputs, trace=False)
    return y


# revision 29
# speedup vs baseline: 1.8314x; 1.0027x over previous
"""Multi-head attention (B=8, T=2048, D=512, H=8) on 8 TRN2 NeuronCores.

Sharding: data-parallel over batch — one batch element per core, no
collectives. Host-side prep (part of shard/unshard): transpose x inputs to
[D, T], cast matmul operands to bf16, pass (1 - mask)^T chunk-major, and
transpose the per-core output y^T back to [T, D].

Per-core algorithm ("transposed flash", everything in one PE tiling mode):
  P1: Q^T = Wq x^T, K^T = Wk x^T (padded per-head into zero-padded 128-row
      tiles), V = x Wv^T (augmented with a ones column per head for the
      softmax denominator).
  P2: per (q-block, head, t2-chunk):
        S^T[t2,q] = Kpad_h^T.T @ Q^T          (PSUM, scale deferred)
        P_raw     = exp(S^T / 8)              (ScalarE, PSUM -> SBUF bf16)
        P         = P_raw * (1-mask)^T        (VectorE; equals reference's
                                               where(mask,-inf) + where(mask,0)
                                               since exp(-1e4) == 0 in f32)
        O_aug^T  += Vaug_h.T @ P              (PSUM accum; row 64 = denom)
      epilogue: recip(denom) -> broadcast -> O^T = O_aug^T[0:64] * recip.
  P3: y^T = Wo^T.T @ O^T (+bo), DMA out.

Biases bq, bk, bo are applied (per-partition fused adds); bv via a
broadcast add on V eviction. No max-subtraction in softmax: scores are
O(6) so exp is safe in f32, matching the reference to ~bf16 accuracy.
"""

import numpy as np
import ml_dtypes

B, T, FDIM, H = 8, 2048, 512, 8
DK = FDIM // H          # 64
NFT = FDIM // 128       # 4 fo-tiles
NCH = T // 128          # 16 t2-chunks
QB = 2                  # q blocks
QBS = T // QB           # 1024
N_CORES = 8

BF16 = ml_dtypes.bfloat16

_cache = {}


def _build_nc():
    import concourse.bass as bass
    import concourse.mybir as mybir
    from concourse import bacc, tile

    f32 = mybir.dt.float32
    bf16 = mybir.dt.bfloat16
    Exp = mybir.ActivationFunctionType.Exp
    Alu = mybir.AluOpType

    nc = bacc.Bacc("TRN2", target_bir_lowering=False, debug=False,
                   num_devices=N_CORES)

    # DRAM I/O (per-core shard shapes)
    xqT = nc.dram_tensor("xqT", [FDIM, T], bf16, kind="ExternalInput")
    xkT = nc.dram_tensor("xkT", [FDIM, T], bf16, kind="ExternalInput")
    xvT = nc.dram_tensor("xvT", [FDIM, T], bf16, kind="ExternalInput")
    wqT = nc.dram_tensor("wqT", [FDIM, FDIM], bf16, kind="ExternalInput")
    wkT = nc.dram_tensor("wkT", [FDIM, FDIM], bf16, kind="ExternalInput")
    wvT = nc.dram_tensor("wvT", [FDIM, FDIM], bf16, kind="ExternalInput")
    woT = nc.dram_tensor("woT", [FDIM, FDIM], bf16, kind="ExternalInput")
    bq = nc.dram_tensor("bq", [FDIM], f32, kind="ExternalInput")
    bk = nc.dram_tensor("bk", [FDIM], f32, kind="ExternalInput")
    bv = nc.dram_tensor("bv", [FDIM], f32, kind="ExternalInput")
    bo = nc.dram_tensor("bo", [FDIM], f32, kind="ExternalInput")
    mbar = nc.dram_tensor("mbar", [NCH, 128, T], bf16, kind="ExternalInput")
    yT = nc.dram_tensor("yT", [FDIM, T], f32, kind="ExternalOutput")
    # DRAM bounce rows for partition-broadcasting softmax reciprocals
    rscratch = nc.dram_tensor("rscratch", [QB * H, QBS], f32)

    import os
    dbg = os.environ.get("KERNEL_DEBUG_DUMPS") == "1"
    if dbg:
        dbg_qt = nc.dram_tensor("dbg_qt", [128, T], bf16, kind="ExternalOutput")
        dbg_kp = nc.dram_tensor("dbg_kp", [2, 128, T], bf16, kind="ExternalOutput")
        dbg_va = nc.dram_tensor("dbg_va", [128, H * (DK + 1)], bf16, kind="ExternalOutput")
        dbg_pm = nc.dram_tensor("dbg_pm", [128, QBS], bf16, kind="ExternalOutput")
        dbg_osb = nc.dram_tensor("dbg_osb", [64, QBS], bf16, kind="ExternalOutput")
        dbg_rb = nc.dram_tensor("dbg_rb", [2, QBS], f32, kind="ExternalOutput")

    with tile.TileContext(nc) as tc:
        with (
            tc.tile_pool(name="consts", bufs=1) as consts,
            tc.tile_pool(name="qt", bufs=1) as qt_pool,
            tc.tile_pool(name="kpad", bufs=1) as kpad_pool,
            tc.tile_pool(name="vaug", bufs=1) as vaug_pool,
            tc.tile_pool(name="osb", bufs=1) as osb_pool,
            tc.tile_pool(name="ysb", bufs=1) as ysb_pool,
        ):
            # ---- consts: weights + biases ----
            wq_sb = [consts.tile([128, FDIM], bf16, tag=f"wq{fc}", name=f"wq{fc}") for fc in range(4)]
            wk_sb = [consts.tile([128, FDIM], bf16, tag=f"wk{fc}", name=f"wk{fc}") for fc in range(4)]
            wv_sb = [consts.tile([128, FDIM], bf16, tag=f"wv{fc}", name=f"wv{fc}") for fc in range(4)]
            wo_sb = [consts.tile([128, FDIM], bf16, tag=f"wo{j}", name=f"wo{j}") for j in range(NFT)]
            for fc in range(4):
                nc.sync.dma_start(out=wq_sb[fc][:], in_=wqT[fc * 128:(fc + 1) * 128, :])
                nc.sync.dma_start(out=wk_sb[fc][:], in_=wkT[fc * 128:(fc + 1) * 128, :])
                nc.sync.dma_start(out=wv_sb[fc][:], in_=wvT[fc * 128:(fc + 1) * 128, :])
            for j in range(NFT):
                nc.sync.dma_start(out=wo_sb[j][:], in_=woT[j * 128:(j + 1) * 128, :])

            bq_sb = consts.tile([128, NFT], f32, tag="bq", name="bq")
            bk_sb = consts.tile([128, NFT], f32, tag="bk", name="bk")
            bo_sb = consts.tile([128, NFT], f32, tag="bo", name="bo")
            for b_dram, b_t in ((bq, bq_sb), (bk, bk_sb), (bo, bo_sb)):
                nc.sync.dma_start(out=b_t[:], in_=b_dram.ap().rearrange("(j p) -> p j", p=128))
            bv_bcast = consts.tile([128, FDIM], f32, tag="bv_bcast", name="bv_bcast")
            nc.sync.dma_start(
                out=bv_bcast[:],
                in_=bv.ap().rearrange("(a f) -> a f", a=1).to_broadcast([128, FDIM]))

            # ---- persistent activation tiles ----
            qT_sb = [qt_pool.tile([128, T], bf16, tag=f"qT{j}", name=f"qT{j}") for j in range(NFT)]
            kpad = [kpad_pool.tile([128, T], bf16, tag=f"kp{h}", name=f"kp{h}") for h in range(H)]
            # zero the unused head-half of each padded K tile, once
            for h in range(H):
                half = slice(64, 128) if h % 2 == 0 else slice(0, 64)
                nc.vector.memset(kpad[h][half, :], 0.0)
            vaug = [vaug_pool.tile([128, H * (DK + 1)], bf16, tag=f"va{tt}", name=f"va{tt}")
                    for tt in range(NCH)]
            # ones column per head in V_aug
            for tt in range(NCH):
                va = vaug[tt][:].rearrange("p (h d) -> p h d", d=DK + 1)
                nc.vector.memset(va[:, :, DK:DK + 1], 1.0)

            o2_sb = {}
            for qb in range(QB):
                for j in range(NFT):
                    o2_sb[(qb, j)] = osb_pool.tile([128, QBS], bf16, tag=f"o2_{qb}_{j}",
                                                   name=f"o2_{qb}_{j}")

            # ============ PSUM pool (8 banks total, shared by phases) ======
            # tag "s":  2 x [128,1024] f32 = 4 banks  (scores / QK-proj)
            # tag o0/o1: 1 x [128,1024]-sized slot each = 4 banks
            #            (attnV accumulators, V-proj, P3 y-tiles)
            with (
                tc.tile_pool(name="xt", bufs=5) as xt_pool,
                tc.tile_pool(name="mask", bufs=16) as mask_pool,
                tc.tile_pool(name="praw", bufs=2) as praw_pool,
                tc.tile_pool(name="pm", bufs=2) as pm_pool,
                tc.tile_pool(name="rb", bufs=1) as rb_pool,
                tc.tile_pool(name="psum", bufs=2, space="PSUM") as psum_pool,
            ):
                def load_xT(xT_dram, tag):
                    tiles = []
                    for fc in range(4):
                        xt = xt_pool.tile([128, T], bf16, tag=tag, bufs=4, name="xt")
                        nc.sync.dma_start(out=xt[:], in_=xT_dram[fc * 128:(fc + 1) * 128, :])
                        tiles.append(xt)
                    return tiles

                def v_proj_tile(tt, ptag):
                    if True:
                        ps = psum_pool.tile([128, 512], mybir.dt.float32,
                                            tag=ptag, bufs=1, name="vp")
                        for fc in range(4):
                            nc.tensor.matmul(
                                ps[:],
                                xts_v[fc][:, tt * 128:(tt + 1) * 128],
                                wv_sb[fc][:],
                                start=(fc == 0), stop=(fc == 3),
                            )
                        va = vaug[tt][:].rearrange("p (h d) -> p h d", d=DK + 1)
                        nc.vector.scalar_tensor_tensor(
                            out=va[:, :, 0:DK],
                            in0=ps[:].rearrange("p (h d) -> p h d", d=DK),
                            scalar=1.0,
                            in1=bv_bcast[:].rearrange("p (h d) -> p h d", d=DK),
                            op0=Alu.mult, op1=Alu.add,
                        )

                def proj_groups(j, xts, w_sb, b_t, dst, slices):
                    for s in slices:
                        ps = psum_pool.tile([128, 512], mybir.dt.float32,
                                            tag="s", name="qkp")
                        for fc in range(4):
                            nc.tensor.matmul(
                                ps[:],
                                w_sb[fc][:, j * 128:(j + 1) * 128],
                                xts[fc][:, s * 512:(s + 1) * 512],
                                start=(fc == 0), stop=(fc == 3),
                            )
                        sl = slice(s * 512, (s + 1) * 512)
                        if dst is not None:
                            nc.vector.tensor_scalar_add(dst[:, sl], ps[:],
                                                        b_t[:, j:j + 1])
                        else:  # K: evict into the two padded per-head tiles
                            nc.vector.tensor_scalar_add(
                                kpad[2 * j][0:64, sl], ps[0:64, :],
                                b_t[0:64, j:j + 1])
                            nc.vector.tensor_scalar_add(
                                kpad[2 * j + 1][64:128, sl], ps[64:128, :],
                                b_t[64:128, j:j + 1])

                def q_proj(j, half):
                    proj_groups(j, xts_q, wq_sb, bq_sb, qT_sb[j],
                                range(2 * half, 2 * half + 2))

                def k_proj(j, half):
                    proj_groups(j, xts_k, wk_sb, bk_sb, None,
                                range(2 * half, 2 * half + 2))

                def p3(qb):
                    qsl = slice(qb * QBS, (qb + 1) * QBS)
                    for i in range(NFT):
                        y_ps = psum_pool.tile([128, QBS], mybir.dt.float32,
                                              tag=f"o{i % 2}", bufs=1, name="y")
                        for s in range(2):
                            for j in range(NFT):
                                nc.tensor.matmul(
                                    y_ps[:, s * 512:(s + 1) * 512],
                                    wo_sb[j][:, i * 128:(i + 1) * 128],
                                    o2_sb[(qb, j)][:, s * 512:(s + 1) * 512],
                                    start=(j == 0), stop=(j == NFT - 1),
                                )
                        y_sb = ysb_pool.tile([128, QBS], mybir.dt.float32, tag="ysb", name="ysb")
                        nc.vector.tensor_scalar_add(y_sb[:], y_ps[:], bo_sb[:, i:i + 1])
                        nc.sync.dma_start(out=yT[i * 128:(i + 1) * 128, qsl], in_=y_sb[:])

                # ---- P1 prefix: V first (every head needs it), then QK j=0
                xts_v = load_xT(xvT, "xq")  # slots reused by xq after v_proj
                for tt in range(NCH):
                    v_proj_tile(tt, f"o{tt % 2}")
                xts_k = load_xT(xkT, "xk")
                xts_q = load_xT(xqT, "xq")
                k_proj(0, 0)
                k_proj(0, 1)
                q_proj(0, 0)

                if dbg:
                    nc.sync.dma_start(out=dbg_qt.ap(), in_=qT_sb[0][:])
                    nc.sync.dma_start(out=dbg_kp.ap()[0], in_=kpad[0][:])
                    nc.sync.dma_start(out=dbg_kp.ap()[1], in_=kpad[1][:])
                    nc.sync.dma_start(out=dbg_va.ap(), in_=vaug[0][:])

                # ---- P2 + P3, with remaining QK projections interleaved ----
                for qb in range(QB):
                    qsl = slice(qb * QBS, (qb + 1) * QBS)
                    mask_t = []
                    for c in range(NCH):
                        mt = mask_pool.tile([128, QBS], bf16, tag="mask", name="mask")
                        nc.sync.dma_start(out=mt[:], in_=mbar[c, :, qsl])
                        mask_t.append(mt)

                    for h in range(H):
                        j = h // 2
                        o_ps = psum_pool.tile([DK + 1, QBS], mybir.dt.float32,
                                              tag=f"o{h % 2}", bufs=1, name="o")
                        for c in range(NCH):
                            s_ps = psum_pool.tile([128, QBS], mybir.dt.float32,
                                                  tag="s", name="s")
                            for s in range(2):
                                nc.tensor.matmul(
                                    s_ps[:, s * 512:(s + 1) * 512],
                                    kpad[h][:, c * 128:(c + 1) * 128],
                                    qT_sb[j][:, qb * QBS + s * 512: qb * QBS + (s + 1) * 512],
                                    start=True, stop=True,
                                )
                            p_raw = praw_pool.tile([128, QBS], bf16, tag="praw", name="praw")
                            nc.scalar.activation(p_raw[:], s_ps[:], Exp,
                                                 bias=0.0, scale=0.125)
                            p_m = pm_pool.tile([128, QBS], bf16, tag="pm", name="pm")
                            nc.vector.tensor_mul(p_m[:], p_raw[:], mask_t[c][:])
                            if dbg and qb == 0 and h == 0 and c == 0:
                                nc.sync.dma_start(out=dbg_pm.ap(), in_=p_m[:])
                            for s in range(2):
                                nc.tensor.matmul(
                                    o_ps[:, s * 512:(s + 1) * 512],
                                    vaug[c][:, h * (DK + 1):(h + 1) * (DK + 1)],
                                    p_m[:, s * 512:(s + 1) * 512],
                                    start=(c == 0), stop=(c == NCH - 1),
                                )
                        # epilogue: divide by the denominator (row DK of o_ps).
                        # reciprocal is ~8 cyc/elem/lane -> split the [1,1024]
                        # row over 8 partitions via SBUF->SBUF DMA; a DRAM
                        # bounce row broadcasts it across partitions 0-63.
                        rb = rb_pool.tile([128, QBS], mybir.dt.float32, tag="rb", name="rb")
                        rbs = rb_pool.tile([8, QBS // 8], mybir.dt.float32, tag="rbs", name="rbs")
                        rbr = rb_pool.tile([8, QBS // 8], mybir.dt.float32, tag="rbr", name="rbr")
                        nc.vector.tensor_copy(rb[64:65, :], o_ps[DK:DK + 1, :])
                        nc.sync.dma_start(out=rbs[:], in_=rb[64:65, :])
                        nc.vector.reciprocal(rbr[:], rbs[:])
                        rrow = rscratch.ap()[qb * H + h: qb * H + h + 1, :]
                        nc.sync.dma_start(out=rrow, in_=rbr[:])
                        nc.sync.dma_start(out=rb[0:64, :],
                                          in_=rrow.to_broadcast([64, QBS]))
                        osm = rb_pool.tile([64, QBS], bf16, tag="osm", bufs=3, name="osm")
                        nc.vector.tensor_mul(osm[:], o_ps[0:DK, :], rb[0:64, :])
                        nc.sync.dma_start(
                            out=o2_sb[(qb, h // 2)][(h % 2) * 64:(h % 2) * 64 + 64, :],
                            in_=osm[:])
                        if dbg and qb == 0 and h == 0:
                            nc.sync.dma_start(out=dbg_rb.ap()[0:1, :], in_=rb[0:1, :])
                            nc.sync.dma_start(out=dbg_rb.ap()[1:2, :], in_=rb[64:65, :])
                            nc.sync.dma_start(out=dbg_osb.ap(), in_=osm[:])

                        # overlap remaining projections with the attention
                        # stream: K(j) fully before head 2j; Q(j) per q-block.
                        steps = ()
                        if qb == 0:
                            steps = [(("k", 1, 0),),
                                     (("k", 1, 1), ("q", 1, 0)),
                                     (("k", 2, 0),), (("k", 2, 1), ("q", 2, 0)),
                                     (("k", 3, 0),), (("k", 3, 1), ("q", 3, 0)),
                                     (("q", 0, 1),), ()][h]
                        else:
                            steps = [(("q", 1, 1),), (("q", 2, 1),),
                                     (("q", 3, 1),)][h] if h < 3 else ()
                        for kind, jj, hh in steps:
                            if kind == "k":
                                k_proj(jj, hh)
                            else:
                                q_proj(jj, hh)
                        if qb == 1 and h == 0:
                            p3(0)


                p3(1)

    nc.compile()
    return nc


def _get_nc():
    if "nc" not in _cache:
        _cache["nc"] = _build_nc()
    return _cache["nc"]


def _make_in_maps(inputs):
    query = np.asarray(inputs["query"], np.float32)
    key = np.asarray(inputs["key"], np.float32)
    value = np.asarray(inputs["value"], np.float32)
    mask = np.asarray(inputs["mask"], bool)
    shared = {
        "wqT": np.ascontiguousarray(np.asarray(inputs["Wq"], np.float32).T).astype(BF16),
        "wkT": np.ascontiguousarray(np.asarray(inputs["Wk"], np.float32).T).astype(BF16),
        "wvT": np.ascontiguousarray(np.asarray(inputs["Wv"], np.float32).T).astype(BF16),
        "woT": np.ascontiguousarray(np.asarray(inputs["Wo"], np.float32).T).astype(BF16),
        "bq": np.asarray(inputs["bq"], np.float32),
        "bk": np.asarray(inputs["bk"], np.float32),
        "bv": np.asarray(inputs["bv"], np.float32),
        "bo": np.asarray(inputs["bo"], np.float32),
    }
    in_maps = []
    for b in range(N_CORES):
        m = dict(shared)
        m["xqT"] = np.ascontiguousarray(query[b].T).astype(BF16)
        m["xkT"] = np.ascontiguousarray(key[b].T).astype(BF16)
        m["xvT"] = np.ascontiguousarray(value[b].T).astype(BF16)
        mb = (~mask[b]).T.astype(BF16)          # (1 - mask)^T, [t2, q]
        m["mbar"] = np.ascontiguousarray(mb.reshape(NCH, 128, T))
        in_maps.append(m)
    return in_maps


def run(inputs, trace=False, **kwargs):
    from concourse.bass_utils import run_bass_kernel_spmd
    nc = _get_nc()
    res = run_bass_kernel_spmd(nc, _make_in_maps(inputs),
                               core_ids=list(range(N_CORES)),
                               trace=trace, **kwargs)
    y = np.stack([np.asarray(res.results[b]["yT"], np.float32).T
                  for b in range(N_CORES)])
    return y, res


def kernel(**inputs) -> np.ndarray:
    y, _ = run(inputs, trace=False)
    return y


# revision 30
# speedup vs baseline: 1.8550x; 1.0129x over previous
"""Multi-head attention (B=8, T=2048, D=512, H=8) on 8 TRN2 NeuronCores.

Sharding: data-parallel over batch — one batch element per core, no
collectives. Host-side prep (part of shard/unshard): transpose x inputs to
[D, T], cast matmul operands to bf16, pass (1 - mask)^T chunk-major, and
transpose the per-core output y^T back to [T, D].

Per-core algorithm ("transposed flash", everything in one PE tiling mode):
  P1: Q^T = Wq x^T, K^T = Wk x^T (padded per-head into zero-padded 128-row
      tiles), V = x Wv^T (augmented with a ones column per head for the
      softmax denominator).
  P2: per (q-block, head, t2-chunk):
        S^T[t2,q] = Kpad_h^T.T @ Q^T          (PSUM, scale deferred)
        P_raw     = exp(S^T / 8)              (ScalarE, PSUM -> SBUF bf16)
        P         = P_raw * (1-mask)^T        (VectorE; equals reference's
                                               where(mask,-inf) + where(mask,0)
                                               since exp(-1e4) == 0 in f32)
        O_aug^T  += Vaug_h.T @ P              (PSUM accum; row 64 = denom)
      epilogue: recip(denom) -> broadcast -> O^T = O_aug^T[0:64] * recip.
  P3: y^T = Wo^T.T @ O^T (+bo), DMA out.

Biases bq, bk, bo are applied (per-partition fused adds); bv via a
broadcast add on V eviction. No max-subtraction in softmax: scores are
O(6) so exp is safe in f32, matching the reference to ~bf16 accuracy.
"""

import numpy as np
import ml_dtypes

B, T, FDIM, H = 8, 2048, 512, 8
DK = FDIM // H          # 64
NFT = FDIM // 128       # 4 fo-tiles
NCH = T // 128          # 16 t2-chunks
QB = 2                  # q blocks
QBS = T // QB           # 1024
N_CORES = 8

BF16 = ml_dtypes.bfloat16

_cache = {}


def _build_nc():
    import concourse.bass as bass
    import concourse.mybir as mybir
    from concourse import bacc, tile

    f32 = mybir.dt.float32
    bf16 = mybir.dt.bfloat16
    Exp = mybir.ActivationFunctionType.Exp
    Alu = mybir.AluOpType

    nc = bacc.Bacc("TRN2", target_bir_lowering=False, debug=False,
                   num_devices=N_CORES)

    # DRAM I/O (per-core shard shapes)
    xqT = nc.dram_tensor("xqT", [FDIM, T], bf16, kind="ExternalInput")
    xkT = nc.dram_tensor("xkT", [FDIM, T], bf16, kind="ExternalInput")
    xvT = nc.dram_tensor("xvT", [FDIM, T], bf16, kind="ExternalInput")
    wqT = nc.dram_tensor("wqT", [FDIM, FDIM], bf16, kind="ExternalInput")
    wkT = nc.dram_tensor("wkT", [FDIM, FDIM], bf16, kind="ExternalInput")
    wvT = nc.dram_tensor("wvT", [FDIM, FDIM], bf16, kind="ExternalInput")
    woT = nc.dram_tensor("woT", [FDIM, FDIM], bf16, kind="ExternalInput")
    bq = nc.dram_tensor("bq", [FDIM], f32, kind="ExternalInput")
    bk = nc.dram_tensor("bk", [FDIM], f32, kind="ExternalInput")
    bv = nc.dram_tensor("bv", [FDIM], f32, kind="ExternalInput")
    bo = nc.dram_tensor("bo", [FDIM], f32, kind="ExternalInput")
    mbar = nc.dram_tensor("mbar", [NCH, 128, T], bf16, kind="ExternalInput")
    yT = nc.dram_tensor("yT", [FDIM, T], f32, kind="ExternalOutput")
    # DRAM bounce rows for partition-broadcasting softmax reciprocals
    rscratch = nc.dram_tensor("rscratch", [QB * H, QBS], f32)

    import os
    dbg = os.environ.get("KERNEL_DEBUG_DUMPS") == "1"
    if dbg:
        dbg_qt = nc.dram_tensor("dbg_qt", [128, T], bf16, kind="ExternalOutput")
        dbg_kp = nc.dram_tensor("dbg_kp", [2, 128, T], bf16, kind="ExternalOutput")
        dbg_va = nc.dram_tensor("dbg_va", [128, H * (DK + 1)], bf16, kind="ExternalOutput")
        dbg_pm = nc.dram_tensor("dbg_pm", [128, QBS], bf16, kind="ExternalOutput")
        dbg_osb = nc.dram_tensor("dbg_osb", [64, QBS], bf16, kind="ExternalOutput")
        dbg_rb = nc.dram_tensor("dbg_rb", [2, QBS], f32, kind="ExternalOutput")

    with tile.TileContext(nc) as tc:
        with (
            tc.tile_pool(name="consts", bufs=1) as consts,
            tc.tile_pool(name="qt", bufs=1) as qt_pool,
            tc.tile_pool(name="kpad", bufs=1) as kpad_pool,
            tc.tile_pool(name="vaug", bufs=1) as vaug_pool,
            tc.tile_pool(name="osb", bufs=1) as osb_pool,
            tc.tile_pool(name="ysb", bufs=1) as ysb_pool,
        ):
            # ---- consts: weights + biases ----
            wq_sb = [consts.tile([128, FDIM], bf16, tag=f"wq{fc}", name=f"wq{fc}") for fc in range(4)]
            wk_sb = [consts.tile([128, FDIM], bf16, tag=f"wk{fc}", name=f"wk{fc}") for fc in range(4)]
            wv_sb = [consts.tile([128, FDIM], bf16, tag=f"wv{fc}", name=f"wv{fc}") for fc in range(4)]
            wo_sb = [consts.tile([128, FDIM], bf16, tag=f"wo{j}", name=f"wo{j}") for j in range(NFT)]
            # wv first: the V projection is the head of the critical path
            for fc in range(4):
                nc.sync.dma_start(out=wv_sb[fc][:], in_=wvT[fc * 128:(fc + 1) * 128, :])
            for fc in range(4):
                nc.sync.dma_start(out=wq_sb[fc][:], in_=wqT[fc * 128:(fc + 1) * 128, :])
                nc.sync.dma_start(out=wk_sb[fc][:], in_=wkT[fc * 128:(fc + 1) * 128, :])
            for j in range(NFT):
                nc.sync.dma_start(out=wo_sb[j][:], in_=woT[j * 128:(j + 1) * 128, :])

            bq_sb = consts.tile([128, NFT], f32, tag="bq", name="bq")
            bk_sb = consts.tile([128, NFT], f32, tag="bk", name="bk")
            bo_sb = consts.tile([128, NFT], f32, tag="bo", name="bo")
            for b_dram, b_t in ((bq, bq_sb), (bk, bk_sb), (bo, bo_sb)):
                nc.sync.dma_start(out=b_t[:], in_=b_dram.ap().rearrange("(j p) -> p j", p=128))
            bv_bcast = consts.tile([128, FDIM], f32, tag="bv_bcast", name="bv_bcast")
            nc.sync.dma_start(
                out=bv_bcast[:],
                in_=bv.ap().rearrange("(a f) -> a f", a=1).to_broadcast([128, FDIM]))

            # ---- persistent activation tiles ----
            qT_sb = [qt_pool.tile([128, T], bf16, tag=f"qT{j}", name=f"qT{j}") for j in range(NFT)]
            kpad = [kpad_pool.tile([128, T], bf16, tag=f"kp{h}", name=f"kp{h}") for h in range(H)]
            # zero the unused head-half of each padded K tile, once
            for h in range(H):
                half = slice(64, 128) if h % 2 == 0 else slice(0, 64)
                nc.vector.memset(kpad[h][half, :], 0.0)
            vaug = [vaug_pool.tile([128, H * (DK + 1)], bf16, tag=f"va{tt}", name=f"va{tt}")
                    for tt in range(NCH)]
            # ones column per head in V_aug
            for tt in range(NCH):
                va = vaug[tt][:].rearrange("p (h d) -> p h d", d=DK + 1)
                nc.vector.memset(va[:, :, DK:DK + 1], 1.0)

            o2_sb = {}
            for qb in range(QB):
                for j in range(NFT):
                    o2_sb[(qb, j)] = osb_pool.tile([128, QBS], bf16, tag=f"o2_{qb}_{j}",
                                                   name=f"o2_{qb}_{j}")

            # ============ PSUM pool (8 banks total, shared by phases) ======
            # tag "s":  2 x [128,1024] f32 = 4 banks  (scores / QK-proj)
            # tag o0/o1: 1 x [128,1024]-sized slot each = 4 banks
            #            (attnV accumulators, V-proj, P3 y-tiles)
            with (
                tc.tile_pool(name="xt", bufs=5) as xt_pool,
                tc.tile_pool(name="mask", bufs=16) as mask_pool,
                tc.tile_pool(name="praw", bufs=2) as praw_pool,
                tc.tile_pool(name="pm", bufs=2) as pm_pool,
                tc.tile_pool(name="rb", bufs=1) as rb_pool,
                tc.tile_pool(name="psum", bufs=2, space="PSUM") as psum_pool,
            ):
                def load_xT(xT_dram, tag):
                    tiles = []
                    for fc in range(4):
                        xt = xt_pool.tile([128, T], bf16, tag=tag, bufs=4, name="xt")
                        nc.sync.dma_start(out=xt[:], in_=xT_dram[fc * 128:(fc + 1) * 128, :])
                        tiles.append(xt)
                    return tiles

                def v_proj_tile(tt, ptag):
                    if True:
                        ps = psum_pool.tile([128, 512], mybir.dt.float32,
                                            tag=ptag, bufs=1, name="vp")
                        for fc in range(4):
                            nc.tensor.matmul(
                                ps[:],
                                xts_v[fc][:, tt * 128:(tt + 1) * 128],
                                wv_sb[fc][:],
                                start=(fc == 0), stop=(fc == 3),
                            )
                        va = vaug[tt][:].rearrange("p (h d) -> p h d", d=DK + 1)
                        nc.vector.scalar_tensor_tensor(
                            out=va[:, :, 0:DK],
                            in0=ps[:].rearrange("p (h d) -> p h d", d=DK),
                            scalar=1.0,
                            in1=bv_bcast[:].rearrange("p (h d) -> p h d", d=DK),
                            op0=Alu.mult, op1=Alu.add,
                        )

                def proj_groups(j, xts, w_sb, b_t, dst, slices):
                    for s in slices:
                        ps = psum_pool.tile([128, 512], mybir.dt.float32,
                                            tag="s", name="qkp")
                        for fc in range(4):
                            nc.tensor.matmul(
                                ps[:],
                                w_sb[fc][:, j * 128:(j + 1) * 128],
                                xts[fc][:, s * 512:(s + 1) * 512],
                                start=(fc == 0), stop=(fc == 3),
                            )
                        sl = slice(s * 512, (s + 1) * 512)
                        if dst is not None:
                            nc.vector.tensor_scalar_add(dst[:, sl], ps[:],
                                                        b_t[:, j:j + 1])
                        else:  # K: evict into the two padded per-head tiles
                            nc.vector.tensor_scalar_add(
                                kpad[2 * j][0:64, sl], ps[0:64, :],
                                b_t[0:64, j:j + 1])
                            nc.vector.tensor_scalar_add(
                                kpad[2 * j + 1][64:128, sl], ps[64:128, :],
                                b_t[64:128, j:j + 1])

                def q_proj(j, half):
                    proj_groups(j, xts_q, wq_sb, bq_sb, qT_sb[j],
                                range(2 * half, 2 * half + 2))

                def k_proj(j, half):
                    proj_groups(j, xts_k, wk_sb, bk_sb, None,
                                range(2 * half, 2 * half + 2))

                def p3(qb):
                    qsl = slice(qb * QBS, (qb + 1) * QBS)
                    for i in range(NFT):
                        y_ps = psum_pool.tile([128, QBS], mybir.dt.float32,
                                              tag=f"o{i % 2}", bufs=1, name="y")
                        for j in range(NFT):
                            for s in range(2):
                                nc.tensor.matmul(
                                    y_ps[:, s * 512:(s + 1) * 512],
                                    wo_sb[j][:, i * 128:(i + 1) * 128],
                                    o2_sb[(qb, j)][:, s * 512:(s + 1) * 512],
                                    start=(j == 0), stop=(j == NFT - 1),
                                )
                        y_sb = ysb_pool.tile([128, QBS], mybir.dt.float32, tag="ysb", name="ysb")
                        nc.vector.tensor_scalar_add(y_sb[:], y_ps[:], bo_sb[:, i:i + 1])
                        nc.sync.dma_start(out=yT[i * 128:(i + 1) * 128, qsl], in_=y_sb[:])

                # ---- P1 prefix: V first (every head needs it), then QK j=0
                xts_v = load_xT(xvT, "xq")  # slots reused by xq after v_proj
                for tt in range(NCH):
                    v_proj_tile(tt, f"o{tt % 2}")
                xts_k = load_xT(xkT, "xk")
                xts_q = load_xT(xqT, "xq")
                k_proj(0, 0)
                k_proj(0, 1)
                q_proj(0, 0)

                if dbg:
                    nc.sync.dma_start(out=dbg_qt.ap(), in_=qT_sb[0][:])
                    nc.sync.dma_start(out=dbg_kp.ap()[0], in_=kpad[0][:])
                    nc.sync.dma_start(out=dbg_kp.ap()[1], in_=kpad[1][:])
                    nc.sync.dma_start(out=dbg_va.ap(), in_=vaug[0][:])

                # ---- P2 + P3, with remaining QK projections interleaved ----
                for qb in range(QB):
                    qsl = slice(qb * QBS, (qb + 1) * QBS)
                    mask_t = []
                    for c in range(NCH):
                        mt = mask_pool.tile([128, QBS], bf16, tag="mask", name="mask")
                        nc.sync.dma_start(out=mt[:], in_=mbar[c, :, qsl])
                        mask_t.append(mt)

                    for h in range(H):
                        j = h // 2
                        o_ps = psum_pool.tile([DK + 1, QBS], mybir.dt.float32,
                                              tag=f"o{h % 2}", bufs=1, name="o")
                        for c in range(NCH):
                            s_ps = psum_pool.tile([128, QBS], mybir.dt.float32,
                                                  tag="s", name="s")
                            for s in range(2):
                                nc.tensor.matmul(
                                    s_ps[:, s * 512:(s + 1) * 512],
                                    kpad[h][:, c * 128:(c + 1) * 128],
                                    qT_sb[j][:, qb * QBS + s * 512: qb * QBS + (s + 1) * 512],
                                    start=True, stop=True,
                                )
                            p_raw = praw_pool.tile([128, QBS], bf16, tag="praw", name="praw")
                            nc.scalar.activation(p_raw[:], s_ps[:], Exp,
                                                 bias=0.0, scale=0.125)
                            p_m = pm_pool.tile([128, QBS], bf16, tag="pm", name="pm")
                            nc.vector.tensor_mul(p_m[:], p_raw[:], mask_t[c][:])
                            if dbg and qb == 0 and h == 0 and c == 0:
                                nc.sync.dma_start(out=dbg_pm.ap(), in_=p_m[:])
                            for s in range(2):
                                nc.tensor.matmul(
                                    o_ps[:, s * 512:(s + 1) * 512],
                                    vaug[c][:, h * (DK + 1):(h + 1) * (DK + 1)],
                                    p_m[:, s * 512:(s + 1) * 512],
                                    start=(c == 0), stop=(c == NCH - 1),
                                )
                        # epilogue: divide by the denominator (row DK of o_ps).
                        # reciprocal is ~8 cyc/elem/lane -> split the [1,1024]
                        # row over 8 partitions via SBUF->SBUF DMA; a DRAM
                        # bounce row broadcasts it across partitions 0-63.
                        rb = rb_pool.tile([128, QBS], mybir.dt.float32, tag="rb", name="rb")
                        rbs = rb_pool.tile([8, QBS // 8], mybir.dt.float32, tag="rbs", name="rbs")
                        rbr = rb_pool.tile([8, QBS // 8], mybir.dt.float32, tag="rbr", name="rbr")
                        nc.vector.tensor_copy(rb[64:65, :], o_ps[DK:DK + 1, :])
                        nc.sync.dma_start(out=rbs[:], in_=rb[64:65, :])
                        nc.vector.reciprocal(rbr[:], rbs[:])
                        rrow = rscratch.ap()[qb * H + h: qb * H + h + 1, :]
                        nc.sync.dma_start(out=rrow, in_=rbr[:])
                        nc.sync.dma_start(out=rb[0:64, :],
                                          in_=rrow.to_broadcast([64, QBS]))
                        osm = rb_pool.tile([64, QBS], bf16, tag="osm", bufs=3, name="osm")
                        nc.vector.tensor_mul(osm[:], o_ps[0:DK, :], rb[0:64, :])
                        nc.sync.dma_start(
                            out=o2_sb[(qb, h // 2)][(h % 2) * 64:(h % 2) * 64 + 64, :],
                            in_=osm[:])
                        if dbg and qb == 0 and h == 0:
                            nc.sync.dma_start(out=dbg_rb.ap()[0:1, :], in_=rb[0:1, :])
                            nc.sync.dma_start(out=dbg_rb.ap()[1:2, :], in_=rb[64:65, :])
                            nc.sync.dma_start(out=dbg_osb.ap(), in_=osm[:])

                        # overlap remaining projections with the attention
                        # stream: K(j) fully before head 2j; Q(j) per q-block.
                        steps = ()
                        if qb == 0:
                            steps = [(("k", 1, 0),),
                                     (("k", 1, 1), ("q", 1, 0)),
                                     (("k", 2, 0),), (("k", 2, 1), ("q", 2, 0)),
                                     (("k", 3, 0),), (("k", 3, 1), ("q", 3, 0)),
                                     (("q", 0, 1),), ()][h]
                        else:
                            steps = [(("q", 1, 1),), (("q", 2, 1),),
                                     (("q", 3, 1),)][h] if h < 3 else ()
                        for kind, jj, hh in steps:
                            if kind == "k":
                                k_proj(jj, hh)
                            else:
                                q_proj(jj, hh)
                        if qb == 1 and h == 0:
                            p3(0)


                p3(1)

    nc.compile()
    return nc


def _get_nc():
    if "nc" not in _cache:
        _cache["nc"] = _build_nc()
    return _cache["nc"]


def _make_in_maps(inputs):
    query = np.asarray(inputs["query"], np.float32)
    key = np.asarray(inputs["key"], np.float32)
    value = np.asarray(inputs["value"], np.float32)
    mask = np.asarray(inputs["mask"], bool)
    shared = {
        "wqT": np.ascontiguousarray(np.asarray(inputs["Wq"], np.float32).T).astype(BF16),
        "wkT": np.ascontiguousarray(np.asarray(inputs["Wk"], np.float32).T).astype(BF16),
        "wvT": np.ascontiguousarray(np.asarray(inputs["Wv"], np.float32).T).astype(BF16),
        "woT": np.ascontiguousarray(np.asarray(inputs["Wo"], np.float32).T).astype(BF16),
        "bq": np.asarray(inputs["bq"], np.float32),
        "bk": np.asarray(inputs["bk"], np.float32),
        "bv": np.asarray(inputs["bv"], np.float32),
        "bo": np.asarray(inputs["bo"], np.float32),
    }
    in_maps = []
    for b in range(N_CORES):
        m = dict(shared)
        m["xqT"] = np.ascontiguousarray(query[b].T).astype(BF16)
        m["xkT"] = np.ascontiguousarray(key[b].T).astype(BF16)
        m["xvT"] = np.ascontiguousarray(value[b].T).astype(BF16)
        mb = (~mask[b]).T.astype(BF16)          # (1 - mask)^T, [t2, q]
        m["mbar"] = np.ascontiguousarray(mb.reshape(NCH, 128, T))
        in_maps.append(m)
    return in_maps


def run(inputs, trace=False, **kwargs):
    from concourse.bass_utils import run_bass_kernel_spmd
    nc = _get_nc()
    res = run_bass_kernel_spmd(nc, _make_in_maps(inputs),
                               core_ids=list(range(N_CORES)),
                               trace=trace, **kwargs)
    y = np.stack([np.asarray(res.results[b]["yT"], np.float32).T
                  for b in range(N_CORES)])
    return y, res


def kernel(**inputs) -> np.ndarray:
    y, _ = run(inputs, trace=False)
    return y
